# revision 5
# baseline (speedup 1.0000x reference)
"""GPT-2 style transformer block on 8 TRN2 NeuronCores.

Sharding: token-data-parallel. Each batch's 2048 tokens are split into 8
chunks of 256; core c owns batch c//4 and chunks {j, 7-j} (j = c%4) so
causal attention work is balanced. QKV/proj/MLP/LN are purely local; the
only collective is one AllGather of (k^T, v) within each 4-core batch
group. Causality is enforced with per-core 0/1 mask tensors so all cores
run one identical SPMD graph (uniform loop bounds; masks zero the
beyond-causal tiles, which also makes the per-core graphs j-independent).

Matmuls run in bf16 (f32 PSUM accumulation); LN/softmax/residuals in f32.
LN affine params are folded into the following matmul weights host-side;
the attention 1/sqrt(hd) scale is folded into w_q; the v-bias is folded
into the proj bias via the softmax-rows-sum-to-one identity. Softmax is
computed without max-subtraction (scores are O(1) here, exp cannot
overflow in f32) as exp(s) normalized by a denominator obtained for free
as an extra ones-column in the av matmul.
"""

import os
import sys

sys.path.insert(0, "/opt/trn_rl_repo")

import numpy as np
import ml_dtypes

import concourse.bass as bass
import concourse.tile as tile
from concourse import bacc, mybir
from concourse.bass_utils import run_bass_kernel_spmd
from concourse.masks import make_identity

F32 = mybir.dt.float32
BF16 = mybir.dt.bfloat16
BF = ml_dtypes.bfloat16

B, T, C, H, HD = 2, 2048, 768, 12, 64
EPS = 1e-5
NCORES = 8
CHUNK = 256            # global chunk size (tokens)
TLOC = 512             # local tokens per core (2 chunks)
NKT = T // 128         # 16 key tiles per batch
KT_Q0, KT_Q1 = 8, 16   # uniform kt bounds for local q-chunk 0 / 1
CC = T // CHUNK        # 8 chunks per batch

# DRAM bounce layout (bf16 elements): k^T block then v block
KT_ELEMS = 6 * 128 * TLOC          # [6 ct][128 p][512 t]
V_ELEMS = 4 * 128 * C              # [4 tt][128 p][768 c]
KV_ELEMS = KT_ELEMS + V_ELEMS

LAST_EXEC_NS = None
LAST_RESULTS = None
_CACHE = {}


def _rank_of_chunk(ck):
    return ck if ck < 4 else 7 - ck


def _loc_of_chunk(ck):
    return 0 if ck < 4 else CHUNK


def _build(add_qk_bias, add_proj_bias, add_fc2_bias):
    nc = bacc.Bacc("TRN2", target_bir_lowering=False, debug=False,
                   num_devices=NCORES)

    x_ext = nc.dram_tensor("x", [TLOC, C], F32, kind="ExternalInput")
    wq_ext = nc.dram_tensor("wq", [C, C], BF16, kind="ExternalInput")
    wk_ext = nc.dram_tensor("wk", [C, C], BF16, kind="ExternalInput")
    wv_ext = nc.dram_tensor("wv", [C, C], BF16, kind="ExternalInput")
    wp_ext = nc.dram_tensor("wp", [12, 64, C], BF16, kind="ExternalInput")
    wfc_ext = nc.dram_tensor("wfc", [C, 4 * C], BF16, kind="ExternalInput")
    wfc2_ext = nc.dram_tensor("wfc2", [4 * C, C], BF16, kind="ExternalInput")
    masks_ext = nc.dram_tensor("masks", [KT_Q0 + KT_Q1, 128, CHUNK], BF16,
                               kind="ExternalInput")
    bqk_ext = nc.dram_tensor("bqk", [2, C], F32, kind="ExternalInput")
    bfc_ext = nc.dram_tensor("bfc", [4 * C], F32, kind="ExternalInput")
    bout_ext = nc.dram_tensor("bout", [2, C], F32, kind="ExternalInput")
    out_ext = nc.dram_tensor("out", [TLOC, C], F32, kind="ExternalOutput")

    with tile.TileContext(nc) as tc:
        with tc.tile_pool(name="dram", bufs=1, space="DRAM") as dram, \
             tc.tile_pool(name="singles", bufs=1) as singles, \
             tc.tile_pool(name="persist", bufs=1) as persist, \
             tc.tile_pool(name="small", bufs=8) as small:

            kv_in = dram.tile([KV_ELEMS], BF16)
            kv_all = dram.tile([4, KV_ELEMS], BF16)

            ident = singles.tile([128, 128], BF16)
            make_identity(nc, ident)
            eps_sb = singles.tile([128, 1], F32)
            nc.vector.memset(eps_sb, EPS)
            ones64 = singles.tile([1, 64], F32)
            nc.vector.memset(ones64, 1.0)

            # resident weights (bf16): [p][6 ct][C]
            wq_sb = persist.tile([128, 6, C], BF16)
            wk_sb = persist.tile([128, 6, C], BF16)
            wv_sb = persist.tile([128, 6, C], BF16)
            wp_sb = persist.tile([64, 12, C], BF16)
            for sb, ext in ((wq_sb, wq_ext), (wk_sb, wk_ext),
                            (wv_sb, wv_ext)):
                nc.sync.dma_start(
                    out=sb, in_=ext.ap().rearrange("(ct p) c -> p ct c", p=128))
            nc.sync.dma_start(
                out=wp_sb, in_=wp_ext.ap().rearrange("h p c -> p h c"))

            bqk_sb = singles.tile([128, 2, 6], F32)
            if add_qk_bias:
                nc.sync.dma_start(
                    out=bqk_sb,
                    in_=bqk_ext.ap().rearrange("b (m p) -> p b m", p=128))
            bfc_sb = singles.tile([128, 24], F32)
            nc.sync.dma_start(
                out=bfc_sb, in_=bfc_ext.ap().rearrange("(m p) -> p m", p=128))
            bout_sb = singles.tile([128, 2, C], F32)
            if add_proj_bias or add_fc2_bias:
                bc = bout_ext.ap()
                nc.sync.dma_start(
                    out=bout_sb,
                    in_=bass.AP(tensor=bc.tensor, offset=bc.offset,
                                ap=[[0, 128], bc.ap[0], bc.ap[1]]))

            masks_sb = persist.tile([128, KT_Q0 + KT_Q1, CHUNK], BF16)
            nc.sync.dma_start(out=masks_sb,
                              in_=masks_ext.ap().rearrange("m p c -> p m c"))

            x_sb = persist.tile([128, 4, C], F32)     # local x, becomes xmid
            hT = persist.tile([128, 6, TLOC], BF16)   # h^T, reused for h2^T
            qT = persist.tile([128, 6, TLOC], BF16)
            kT = persist.tile([128, 6, TLOC], BF16)
            yT_all = persist.tile([64, 2, 12, CHUNK], BF16)

            def layernorm_to(pool, xt, dst, tagsuf):
                stats = pool.tile([128, 3, 6], F32, tag="st" + tagsuf)
                for sg in range(3):
                    nc.vector.bn_stats(out=stats[:, sg, :],
                                       in_=xt[:, sg * 256:(sg + 1) * 256])
                mv = pool.tile([128, 2], F32, tag="mv" + tagsuf)
                nc.vector.bn_aggr(out=mv, in_=stats)
                nc.scalar.activation(out=mv[:, 1:2], in_=mv[:, 1:2],
                                     func=mybir.ActivationFunctionType.Sqrt,
                                     bias=eps_sb)
                nc.vector.reciprocal(out=mv[:, 1:2], in_=mv[:, 1:2])
                nc.vector.tensor_scalar(out=dst, in0=xt,
                                        scalar1=mv[:, 0:1], scalar2=mv[:, 1:2],
                                        op0=mybir.AluOpType.subtract,
                                        op1=mybir.AluOpType.mult)

            # ---------------- LN1 + transpose + QKV + AG ----------------
            with tc.tile_pool(name="ln", bufs=3) as lnp, \
                 tc.tile_pool(name="tp", bufs=2, space="PSUM") as tpp, \
                 tc.tile_pool(name="qkp", bufs=2, space="PSUM") as qkp, \
                 tc.tile_pool(name="vp", bufs=2, space="PSUM") as vpp, \
                 tc.tile_pool(name="vsb", bufs=1) as vsbp:

                for t in range(4):
                    nc.sync.dma_start(out=x_sb[:, t, :],
                                      in_=x_ext[t * 128:(t + 1) * 128, :])
                    xn = lnp.tile([128, C], BF16, tag="xn")
                    layernorm_to(lnp, x_sb[:, t, :], xn, "1")
                    for ct in range(6):
                        pt = tpp.tile([128, 128], BF16, tag="tp")
                        nc.tensor.transpose(
                            pt, xn[:, ct * 128:(ct + 1) * 128], ident)
                        nc.vector.tensor_copy(
                            hT[:, ct, t * 128:(t + 1) * 128], pt)

                # k^T first (feeds the collective), then v, then q^T
                for m in range(6):
                    ps = qkp.tile([128, TLOC], F32, tag="qk")
                    for k in range(6):
                        nc.tensor.matmul(
                            ps, lhsT=wk_sb[:, k, m * 128:(m + 1) * 128],
                            rhs=hT[:, k, :], start=(k == 0), stop=(k == 5))
                    if add_qk_bias:
                        nc.vector.tensor_scalar_add(
                            out=kT[:, m, :], in0=ps,
                            scalar1=bqk_sb[:, 1, m:m + 1])
                    else:
                        nc.vector.tensor_copy(kT[:, m, :], ps)
                kv_kT = kv_in[0:KT_ELEMS].rearrange(
                    "(ct p t) -> p ct t", p=128, t=TLOC)
                nc.sync.dma_start(out=kv_kT, in_=kT)

                v_sb = vsbp.tile([128, 4, C], BF16)
                for tt in range(4):
                    pv = vpp.tile([128, C], F32, tag="v")
                    for k in range(6):
                        nc.tensor.matmul(
                            pv[:, 0:512],
                            lhsT=hT[:, k, tt * 128:(tt + 1) * 128],
                            rhs=wv_sb[:, k, 0:512],
                            start=(k == 0), stop=(k == 5))
                        nc.tensor.matmul(
                            pv[:, 512:768],
                            lhsT=hT[:, k, tt * 128:(tt + 1) * 128],
                            rhs=wv_sb[:, k, 512:768],
                            start=(k == 0), stop=(k == 5))
                    nc.vector.tensor_copy(v_sb[:, tt, :], pv)
                kv_v = kv_in[KT_ELEMS:KV_ELEMS].rearrange(
                    "(tt p c) -> p tt c", p=128, c=C)
                nc.sync.dma_start(out=kv_v, in_=v_sb)

                nc.gpsimd.collective_compute(
                    "AllGather", mybir.AluOpType.bypass,
                    replica_groups=[[0, 1, 2, 3], [4, 5, 6, 7]],
                    ins=[kv_in[:].opt()],
                    outs=[kv_all[:].opt()])

                for m in range(6):
                    ps = qkp.tile([128, TLOC], F32, tag="qk")
                    for k in range(6):
                        nc.tensor.matmul(
                            ps, lhsT=wq_sb[:, k, m * 128:(m + 1) * 128],
                            rhs=hT[:, k, :], start=(k == 0), stop=(k == 5))
                    if add_qk_bias:
                        nc.vector.tensor_scalar_add(
                            out=qT[:, m, :], in0=ps,
                            scalar1=bqk_sb[:, 0, m:m + 1])
                    else:
                        nc.vector.tensor_copy(qT[:, m, :], ps)

            # ---------------- attention ----------------
            with tc.tile_pool(name="kch", bufs=1) as kchp, \
                 tc.tile_pool(name="vaug", bufs=1) as vaugp, \
                 tc.tile_pool(name="ep", bufs=2, space="PSUM") as epp, \
                 tc.tile_pool(name="avp", bufs=2, space="PSUM") as avpp, \
                 tc.tile_pool(name="bcp", bufs=2, space="PSUM") as bcpp, \
                 tc.tile_pool(name="esb", bufs=3) as esbp:

                k_ch = kchp.tile([128, CC, 6, CHUNK], BF16)
                v_aug = vaugp.tile([128, NKT, 12 * 65], BF16)

                for ck in range(CC):
                    r = _rank_of_chunk(ck)
                    loc = _loc_of_chunk(ck)
                    src = kv_all[r, 0:KT_ELEMS].rearrange(
                        "(ct p t) -> p ct t", p=128, t=TLOC)[:, :, loc:loc + CHUNK]
                    nc.sync.dma_start(out=k_ch[:, ck, :, :], in_=src)
                for kt in range(NKT):
                    ck = kt // 2
                    r = _rank_of_chunk(ck)
                    lt = kt % 2 + (2 if ck >= 4 else 0)
                    src = kv_all[r, KT_ELEMS:KV_ELEMS].rearrange(
                        "(tt p c) -> tt p c", p=128, c=C)[lt].rearrange(
                        "p (h e) -> p h e", e=64)
                    dst = v_aug[:, kt, :].rearrange("p (h e) -> p h e", e=65)
                    nc.sync.dma_start(out=dst[:, :, 0:64], in_=src)
                    nc.vector.memset(dst[:, :, 64:65], 1.0)

                def finalize_head(qc, h, pav):
                    recip = small.tile([1, CHUNK], F32, tag="recip")
                    nc.vector.reciprocal(recip, pav[64:65, :])
                    pb = bcpp.tile([64, CHUNK], F32, tag="bc")
                    nc.tensor.matmul(pb, lhsT=ones64, rhs=recip,
                                     start=True, stop=True)
                    b_sb = small.tile([64, CHUNK], F32, tag="bsb")
                    nc.vector.tensor_copy(b_sb, pb)
                    nc.vector.tensor_mul(yT_all[:, qc, h, :], pav[0:64, :], b_sb)

                for qc in range(2):
                    n_kt = KT_Q0 if qc == 0 else KT_Q1
                    moff = 0 if qc == 0 else KT_Q0
                    groups = [(g, min(4, n_kt - g)) for g in range(0, n_kt, 4)]
                    pavs = {}
                    pend = None
                    for h in range(12):
                        q_ap = qT[(h % 2) * 64:(h % 2) * 64 + 64, h // 2,
                                  qc * CHUNK:(qc + 1) * CHUNK]
                        pavs[h] = avpp.tile([65, CHUNK], F32, tag="av",
                                            name=f"pav_{qc}_{h}")
                        for (g, gn) in groups:
                            pe = epp.tile([128, 4 * CHUNK], F32, tag="e")
                            for i in range(gn):
                                kt = g + i
                                k_ap = k_ch[(h % 2) * 64:(h % 2) * 64 + 64,
                                            kt // 2, h // 2,
                                            (kt % 2) * 128:(kt % 2) * 128 + 128]
                                nc.tensor.matmul(
                                    pe[:, i * CHUNK:(i + 1) * CHUNK],
                                    lhsT=k_ap, rhs=q_ap, start=True, stop=True)
                            e_sb = esbp.tile([128, 4 * CHUNK], BF16, tag="esb")
                            nc.scalar.activation(
                                out=e_sb[:, 0:gn * CHUNK],
                                in_=pe[:, 0:gn * CHUNK],
                                func=mybir.ActivationFunctionType.Exp)
                            for i in range(gn):
                                kt = g + i
                                nc.vector.tensor_mul(
                                    e_sb[:, i * CHUNK:(i + 1) * CHUNK],
                                    e_sb[:, i * CHUNK:(i + 1) * CHUNK],
                                    masks_sb[:, moff + kt, :])
                            # drain previous group's av matmuls (1-group skew
                            # so PE never waits on the exp just issued)
                            if pend is not None:
                                _emit_av(nc, pend, pavs, n_kt, v_aug)
                                if pend[2] + pend[3] == n_kt:
                                    finalize_head(qc, pend[0], pavs[pend[0]])
                                    del pavs[pend[0]]
                            pend = (h, e_sb, g, gn)
                    if pend is not None:
                        _emit_av(nc, pend, pavs, n_kt, v_aug)
                        finalize_head(qc, pend[0], pavs[pend[0]])
                        del pavs[pend[0]]
                        pend = None

            # ---------------- proj + residual + LN2 ----------------
            with tc.tile_pool(name="pp", bufs=2, space="PSUM") as ppp, \
                 tc.tile_pool(name="ln2", bufs=3) as ln2p, \
                 tc.tile_pool(name="tp2", bufs=2, space="PSUM") as tpp2:

                xn2s = []
                for t in range(4):
                    qc, tt = t // 2, t % 2
                    pp = ppp.tile([128, C], F32, tag="pp")
                    for h in range(12):
                        y_ap = yT_all[:, qc, h, tt * 128:(tt + 1) * 128]
                        w0 = wp_sb[:, h, 0:512]
                        w1 = wp_sb[:, h, 512:768]
                        nc.tensor.matmul(pp[:, 0:512], lhsT=y_ap, rhs=w0,
                                         start=(h == 0), stop=(h == 11))
                        nc.tensor.matmul(pp[:, 512:768], lhsT=y_ap, rhs=w1,
                                         start=(h == 0), stop=(h == 11))
                    nc.vector.tensor_add(x_sb[:, t, :], x_sb[:, t, :], pp)
                    if add_proj_bias:
                        nc.vector.tensor_add(x_sb[:, t, :], x_sb[:, t, :],
                                             bout_sb[:, 0, :])
                    xn2 = ln2p.tile([128, C], BF16, tag="xn2")
                    layernorm_to(ln2p, x_sb[:, t, :], xn2, "2")
                    xn2s.append(xn2)
                for t in range(4):
                    for ct in range(6):
                        pt = tpp2.tile([128, 128], BF16, tag="tp2")
                        nc.tensor.transpose(
                            pt, xn2s[t][:, ct * 128:(ct + 1) * 128], ident)
                        nc.vector.tensor_copy(
                            hT[:, ct, t * 128:(t + 1) * 128], pt)

            # ---------------- MLP ----------------
            with tc.tile_pool(name="mlp", bufs=1) as mlpp, \
                 tc.tile_pool(name="wfc", bufs=3) as wfcp, \
                 tc.tile_pool(name="wfc2", bufs=3) as wfc2p, \
                 tc.tile_pool(name="osb", bufs=3) as osbp:

                gT = mlpp.tile([128, 24, TLOC], BF16)
                wfc_t = wfc_ext.ap().rearrange("(k p) n -> p k n", p=128)
                with tc.tile_pool(name="fcp", bufs=2, space="PSUM") as fcpp:
                    for m in range(24):
                        wt = wfcp.tile([128, 6, 128], BF16, tag="wfc")
                        nc.sync.dma_start(
                            out=wt, in_=wfc_t[:, :, m * 128:(m + 1) * 128])
                        pf = fcpp.tile([128, TLOC], F32, tag="fc")
                        for k in range(6):
                            nc.tensor.matmul(pf, lhsT=wt[:, k, :],
                                             rhs=hT[:, k, :],
                                             start=(k == 0), stop=(k == 5))
                        nc.scalar.activation(
                            out=gT[:, m, :], in_=pf,
                            func=mybir.ActivationFunctionType.Gelu_apprx_tanh,
                            bias=bfc_sb[:, m:m + 1])

                wfc2_t = wfc2_ext.ap().rearrange("(k p) n -> k p n", p=128)
                with tc.tile_pool(name="f2p", bufs=1, space="PSUM") as f2pp:
                    pf2s = [f2pp.tile([128, C], F32, tag=f"f2_{t}",
                                      name=f"pf2_{t}")
                            for t in range(4)]
                    for k in range(24):
                        wt2 = wfc2p.tile([128, C], BF16, tag="wfc2")
                        nc.sync.dma_start(out=wt2, in_=wfc2_t[k])
                        for t in range(4):
                            nc.tensor.matmul(
                                pf2s[t][:, 0:512],
                                lhsT=gT[:, k, t * 128:(t + 1) * 128],
                                rhs=wt2[:, 0:512],
                                start=(k == 0), stop=(k == 23))
                            nc.tensor.matmul(
                                pf2s[t][:, 512:768],
                                lhsT=gT[:, k, t * 128:(t + 1) * 128],
                                rhs=wt2[:, 512:768],
                                start=(k == 0), stop=(k == 23))
                    for t in range(4):
                        o_sb = osbp.tile([128, C], F32, tag="osb")
                        nc.vector.tensor_add(o_sb, x_sb[:, t, :], pf2s[t])
                        if add_fc2_bias:
                            nc.vector.tensor_add(o_sb, o_sb, bout_sb[:, 1, :])
                        nc.sync.dma_start(
                            out=out_ext[t * 128:(t + 1) * 128, :], in_=o_sb)

    nc.compile()
    return nc


def _emit_av(nc, pend, pavs, n_kt, v_aug):
    h, e_sb, g, gn = pend
    pav = pavs[h]
    for i in range(gn):
        kt = g + i
        nc.tensor.matmul(pav,
                         lhsT=v_aug[:, kt, h * 65:(h + 1) * 65],
                         rhs=e_sb[:, i * CHUNK:(i + 1) * CHUNK],
                         start=(kt == 0), stop=(kt == n_kt - 1))


def _preprocess(inputs):
    f = lambda k: np.asarray(inputs[k], np.float32)
    x = f("x"); w_attn = f("w_attn"); b_attn = f("b_attn")
    w_proj = f("w_proj"); b_proj = f("b_proj")
    w_fc = f("w_fc"); b_fc = f("b_fc"); w_fc2 = f("w_fc2"); b_fc2 = f("b_fc2")
    ln1_g = f("ln1_g"); ln1_b = f("ln1_b"); ln2_g = f("ln2_g"); ln2_b = f("ln2_b")

    w_attn_eff = ln1_g[:, None] * w_attn
    b_attn_eff = b_attn + ln1_b @ w_attn
    s = 1.0 / np.sqrt(HD)
    w_q = w_attn_eff[:, 0:C] * s
    w_k = w_attn_eff[:, C:2 * C]
    w_v = w_attn_eff[:, 2 * C:3 * C]
    b_q = b_attn_eff[0:C] * s
    b_k = b_attn_eff[C:2 * C]
    b_v = b_attn_eff[2 * C:3 * C]
    b_proj_eff = b_proj + b_v @ w_proj
    w_fc_eff = ln2_g[:, None] * w_fc
    b_fc_eff = b_fc + ln2_b @ w_fc

    wq16 = np.ascontiguousarray(w_q.astype(BF))
    wk16 = np.ascontiguousarray(w_k.astype(BF))
    wv16 = np.ascontiguousarray(w_v.astype(BF))
    wp16 = np.ascontiguousarray(w_proj.reshape(12, 64, C).astype(BF))
    wfc16 = np.ascontiguousarray(w_fc_eff.astype(BF))
    wfc216 = np.ascontiguousarray(w_fc2.astype(BF))

    bqk = np.stack([b_q, b_k]).astype(np.float32)
    bout = np.stack([b_proj_eff, b_fc2]).astype(np.float32)

    flags = (bool(np.any(bqk != 0)), bool(np.any(b_proj_eff != 0)),
             bool(np.any(b_fc2 != 0)))

    # per-core-group masks [24, 128, 256]: slot m<8 -> qc0 kt=m; else qc1
    masks = np.zeros((4, KT_Q0 + KT_Q1, 128, CHUNK), np.float32)
    kpos = np.arange(128)
    qpos = np.arange(CHUNK)
    for j in range(4):
        for qc, gq in ((0, j), (1, 7 - j)):
            n_kt = KT_Q0 if qc == 0 else KT_Q1
            moff = 0 if qc == 0 else KT_Q0
            for kt in range(n_kt):
                gk = kt * 128 + kpos[:, None]
                gquv = gq * CHUNK + qpos[None, :]
                masks[j, moff + kt] = (gquv >= gk)
    masks16 = masks.astype(BF)

    in_maps = []
    for c in range(NCORES):
        b, j = c // 4, c % 4
        x_loc = np.concatenate(
            [x[b, j * CHUNK:(j + 1) * CHUNK],
             x[b, (7 - j) * CHUNK:(8 - j) * CHUNK]]).astype(np.float32)
        in_maps.append({
            "x": np.ascontiguousarray(x_loc),
            "wq": wq16, "wk": wk16, "wv": wv16, "wp": wp16,
            "wfc": wfc16, "wfc2": wfc216,
            "masks": np.ascontiguousarray(masks16[j]),
            "bqk": bqk, "bfc": b_fc_eff.astype(np.float32), "bout": bout,
        })
    return in_maps, flags


def kernel(**inputs):
    global LAST_EXEC_NS, LAST_RESULTS
    in_maps, flags = _preprocess(inputs)
    if flags not in _CACHE:
        _CACHE[flags] = _build(*flags)
    nc = _CACHE[flags]
    trace = bool(os.environ.get("BASS_KERNEL_TRACE"))
    res = run_bass_kernel_spmd(nc, in_maps, core_ids=list(range(NCORES)),
                               trace=trace)
    LAST_EXEC_NS = res.exec_time_ns
    LAST_RESULTS = res
    out = np.empty((B, T, C), np.float32)
    for c in range(NCORES):
        b, j = c // 4, c % 4
        o = res.results[c]["out"]
        out[b, j * CHUNK:(j + 1) * CHUNK] = o[0:CHUNK]
        out[b, (7 - j) * CHUNK:(8 - j) * CHUNK] = o[CHUNK:TLOC]
    return out


# revision 14
# speedup vs baseline: 1.0781x; 1.0781x over previous
"""GPT-2 style transformer block on 8 TRN2 NeuronCores.

Sharding: token-data-parallel. Each batch's 2048 tokens are split into 8
chunks of 256; core c owns batch c//4 and chunks {j, 7-j} (j = c%4) so
causal attention work is balanced. QKV/proj/MLP/LN are purely local; the
only collectives are two AllGathers (k^T, then v) within each 4-core
batch group. Causality is enforced with per-core 0/1 mask tensors so all
cores run one identical SPMD graph (uniform loop bounds; masks zero the
beyond-causal tiles, which also makes the per-core graphs j-independent).

Matmuls run in bf16 (f32 PSUM accumulation); LN/softmax/residuals in f32.
LN affine params are folded into the following matmul weights host-side;
the attention 1/sqrt(hd) scale is folded into w_q; the v-bias is folded
into the proj bias via the softmax-rows-sum-to-one identity. Softmax is
computed without max-subtraction (scores are O(1) here, exp cannot
overflow in f32) as exp(s) normalized by a denominator obtained for free
as an extra ones-column in the av matmul. Both local q-chunks share one
[65, 512] av accumulator per head; key tiles 0..7 are scored against all
512 local queries in one matmul, tiles 8..15 only against q-chunk 1.
"""

import os
import sys

sys.path.insert(0, "/opt/trn_rl_repo")

import numpy as np
import ml_dtypes

import concourse.bass as bass
import concourse.tile as tile
from concourse import bacc, mybir
from concourse.bass_utils import run_bass_kernel_spmd
from concourse.masks import make_identity

F32 = mybir.dt.float32
BF16 = mybir.dt.bfloat16
BF = ml_dtypes.bfloat16

B, T, C, H, HD = 2, 2048, 768, 12, 64
EPS = 1e-5
NCORES = 8
CHUNK = 256            # global chunk size (tokens)
TLOC = 512             # local tokens per core (2 chunks)
NKT = T // 128         # 16 key tiles per batch
CC = T // CHUNK        # 8 chunks per batch

# e-slot layout: kt<8 -> 512 wide (both q-chunks), kt>=8 -> 256 (q-chunk 1)
def _slot_off(kt):
    return kt * 512 if kt < 8 else 4096 + (kt - 8) * 256


def _slot_w(kt):
    return 512 if kt < 8 else 256


MASK_W = 8 * 512 + 8 * 256   # 6144
# exp groups: contiguous 1024-col spans of the slot layout
GROUPS = [(0, 2), (2, 2), (4, 2), (6, 2), (8, 4), (12, 4)]

KT_ELEMS = 6 * 128 * TLOC          # k^T bounce: [6 ct][128 p][512 t]
V_ELEMS = 4 * 128 * C              # v bounce:   [4 tt][128 p][768 c]

LAST_EXEC_NS = None
LAST_RESULTS = None
_CACHE = {}


def _rank_of_chunk(ck):
    return ck if ck < 4 else 7 - ck


def _loc_of_chunk(ck):
    return 0 if ck < 4 else CHUNK


def _build(add_qk_bias, add_proj_bias, add_fc2_bias):
    nc = bacc.Bacc("TRN2", target_bir_lowering=False, debug=False,
                   num_devices=NCORES)

    x_ext = nc.dram_tensor("x", [TLOC, C], F32, kind="ExternalInput")
    wq_ext = nc.dram_tensor("wq", [C, C], BF16, kind="ExternalInput")
    wk_ext = nc.dram_tensor("wk", [C, C], BF16, kind="ExternalInput")
    wv_ext = nc.dram_tensor("wv", [C, C], BF16, kind="ExternalInput")
    wp_ext = nc.dram_tensor("wp", [12, 64, C], BF16, kind="ExternalInput")
    wfc_ext = nc.dram_tensor("wfc", [C, 4 * C], BF16, kind="ExternalInput")
    wfc2_ext = nc.dram_tensor("wfc2", [4 * C, C], BF16, kind="ExternalInput")
    masks_ext = nc.dram_tensor("masks", [128, MASK_W], BF16,
                               kind="ExternalInput")
    bqk_ext = nc.dram_tensor("bqk", [2, C], F32, kind="ExternalInput")
    bfc_ext = nc.dram_tensor("bfc", [4 * C], F32, kind="ExternalInput")
    bout_ext = nc.dram_tensor("bout", [2, C], F32, kind="ExternalInput")
    out_ext = nc.dram_tensor("out", [TLOC, C], F32, kind="ExternalOutput")

    with tile.TileContext(nc) as tc:
        with tc.tile_pool(name="dram", bufs=1, space="DRAM") as dram, \
             tc.tile_pool(name="singles", bufs=1) as singles, \
             tc.tile_pool(name="persist", bufs=1) as persist, \
             tc.tile_pool(name="small", bufs=3) as small:

            kvk_in = dram.tile([KT_ELEMS], BF16)
            kvk_all = dram.tile([4, KT_ELEMS], BF16)
            kvv_in = dram.tile([V_ELEMS], BF16)
            kvv_all = dram.tile([4, V_ELEMS], BF16)

            ident = singles.tile([128, 128], BF16)
            make_identity(nc, ident)
            eps_sb = singles.tile([128, 1], F32)
            nc.vector.memset(eps_sb, EPS)
            ones64 = singles.tile([1, 64], F32)
            nc.vector.memset(ones64, 1.0)

            wq_sb = persist.tile([128, 6, C], BF16)
            wk_sb = persist.tile([128, 6, C], BF16)
            wv_sb = persist.tile([128, 6, C], BF16)
            wp_sb = persist.tile([64, 12, C], BF16)
            for sb, ext in ((wq_sb, wq_ext), (wk_sb, wk_ext),
                            (wv_sb, wv_ext)):
                nc.sync.dma_start(
                    out=sb, in_=ext.ap().rearrange("(ct p) c -> p ct c", p=128))
            nc.sync.dma_start(
                out=wp_sb, in_=wp_ext.ap().rearrange("h p c -> p h c"))

            bqk_sb = singles.tile([128, 2, 6], F32)
            if add_qk_bias:
                nc.sync.dma_start(
                    out=bqk_sb,
                    in_=bqk_ext.ap().rearrange("b (m p) -> p b m", p=128))
            bfc_sb = singles.tile([128, 24], F32)
            nc.sync.dma_start(
                out=bfc_sb, in_=bfc_ext.ap().rearrange("(m p) -> p m", p=128))
            bout_sb = singles.tile([128, 2, C], F32)
            if add_proj_bias or add_fc2_bias:
                bc = bout_ext.ap()
                nc.sync.dma_start(
                    out=bout_sb,
                    in_=bass.AP(tensor=bc.tensor, offset=bc.offset,
                                ap=[[0, 128], bc.ap[0], bc.ap[1]]))

            masks_sb = persist.tile([128, MASK_W], BF16)
            nc.sync.dma_start(out=masks_sb, in_=masks_ext.ap())

            x_sb = persist.tile([128, 4, C], F32)     # local x, becomes xmid
            hT = persist.tile([128, 6, TLOC], BF16)   # h^T, reused for h2^T
            qT = persist.tile([128, 6, TLOC], BF16)
            yT_all = persist.tile([64, 12, TLOC], BF16)

            def layernorm_to(pool, xt, dst, tagsuf):
                stats = pool.tile([128, 3, 6], F32, tag="st" + tagsuf,
                                  name="st" + tagsuf)
                for sg in range(3):
                    nc.vector.bn_stats(out=stats[:, sg, :],
                                       in_=xt[:, sg * 256:(sg + 1) * 256])
                mv = pool.tile([128, 2], F32, tag="mv" + tagsuf,
                               name="mv" + tagsuf)
                nc.vector.bn_aggr(out=mv, in_=stats)
                nc.scalar.activation(out=mv[:, 1:2], in_=mv[:, 1:2],
                                     func=mybir.ActivationFunctionType.Sqrt,
                                     bias=eps_sb)
                nc.vector.reciprocal(out=mv[:, 1:2], in_=mv[:, 1:2])
                nc.vector.tensor_scalar(out=dst, in0=xt,
                                        scalar1=mv[:, 0:1], scalar2=mv[:, 1:2],
                                        op0=mybir.AluOpType.subtract,
                                        op1=mybir.AluOpType.mult)

            # ---------------- LN1 + transpose + QKV + AGs ----------------
            with tc.tile_pool(name="ln", bufs=3) as lnp, \
                 tc.tile_pool(name="tp", bufs=2, space="PSUM") as tpp, \
                 tc.tile_pool(name="qkp", bufs=2, space="PSUM") as qkp, \
                 tc.tile_pool(name="vp", bufs=2, space="PSUM") as vpp, \
                 tc.tile_pool(name="vsb", bufs=1) as vsbp:

                kT = vsbp.tile([128, 6, TLOC], BF16)
                for t in range(4):
                    nc.sync.dma_start(out=x_sb[:, t, :],
                                      in_=x_ext[t * 128:(t + 1) * 128, :])
                    xn = lnp.tile([128, C], BF16, tag="xn")
                    layernorm_to(lnp, x_sb[:, t, :], xn, "1")
                    for ct in range(6):
                        pt = tpp.tile([128, 128], BF16, tag="tp")
                        nc.tensor.transpose(
                            pt, xn[:, ct * 128:(ct + 1) * 128], ident)
                        nc.vector.tensor_copy(
                            hT[:, ct, t * 128:(t + 1) * 128], pt)

                # k^T first: it feeds the first collective
                for m in range(6):
                    ps = qkp.tile([128, TLOC], F32, tag="qk")
                    for k in range(6):
                        nc.tensor.matmul(
                            ps, lhsT=wk_sb[:, k, m * 128:(m + 1) * 128],
                            rhs=hT[:, k, :], start=(k == 0), stop=(k == 5))
                    if add_qk_bias:
                        nc.vector.tensor_scalar_add(
                            out=kT[:, m, :], in0=ps,
                            scalar1=bqk_sb[:, 1, m:m + 1])
                    else:
                        nc.vector.tensor_copy(kT[:, m, :], ps)
                nc.sync.dma_start(
                    out=kvk_in[:].rearrange("(ct p t) -> p ct t", p=128, t=TLOC),
                    in_=kT)
                nc.gpsimd.collective_compute(
                    "AllGather", mybir.AluOpType.bypass,
                    replica_groups=[[0, 1, 2, 3], [4, 5, 6, 7]],
                    ins=[kvk_in[:].opt()], outs=[kvk_all[:].opt()])

                v_sb = vsbp.tile([128, 4, C], BF16)
                for tt in range(4):
                    pv = vpp.tile([128, C], F32, tag="v")
                    for k in range(6):
                        nc.tensor.matmul(
                            pv[:, 0:512],
                            lhsT=hT[:, k, tt * 128:(tt + 1) * 128],
                            rhs=wv_sb[:, k, 0:512],
                            start=(k == 0), stop=(k == 5))
                        nc.tensor.matmul(
                            pv[:, 512:768],
                            lhsT=hT[:, k, tt * 128:(tt + 1) * 128],
                            rhs=wv_sb[:, k, 512:768],
                            start=(k == 0), stop=(k == 5))
                    nc.vector.tensor_copy(v_sb[:, tt, :], pv)
                nc.sync.dma_start(
                    out=kvv_in[:].rearrange("(tt p c) -> p tt c", p=128, c=C),
                    in_=v_sb)
                nc.gpsimd.collective_compute(
                    "AllGather", mybir.AluOpType.bypass,
                    replica_groups=[[0, 1, 2, 3], [4, 5, 6, 7]],
                    ins=[kvv_in[:].opt()], outs=[kvv_all[:].opt()])

                for m in range(6):
                    ps = qkp.tile([128, TLOC], F32, tag="qk")
                    for k in range(6):
                        nc.tensor.matmul(
                            ps, lhsT=wq_sb[:, k, m * 128:(m + 1) * 128],
                            rhs=hT[:, k, :], start=(k == 0), stop=(k == 5))
                    if add_qk_bias:
                        nc.vector.tensor_scalar_add(
                            out=qT[:, m, :], in0=ps,
                            scalar1=bqk_sb[:, 0, m:m + 1])
                    else:
                        nc.vector.tensor_copy(qT[:, m, :], ps)

            # ---------------- attention ----------------
            with tc.tile_pool(name="kch", bufs=1) as kchp, \
                 tc.tile_pool(name="vch", bufs=1) as vchp, \
                 tc.tile_pool(name="vaug", bufs=1) as vaugp, \
                 tc.tile_pool(name="ep", bufs=2, space="PSUM") as epp, \
                 tc.tile_pool(name="avp", bufs=2, space="PSUM") as avpp, \
                 tc.tile_pool(name="bcp", bufs=2, space="PSUM") as bcpp, \
                 tc.tile_pool(name="esb", bufs=3) as esbp:

                # gathered k: [r][6 ct][128][512]; v: [r][4 lt][128][768]
                k_ch = kchp.tile([128, 4, 6, TLOC], BF16)
                v_ch = vchp.tile([128, 4, 4, C], BF16)
                v_aug = vaugp.tile([128, NKT, 12 * 65], BF16)

                for r in range(4):
                    nc.sync.dma_start(
                        out=k_ch[:, r, :, :],
                        in_=kvk_all[r].rearrange("(ct p t) -> p ct t",
                                                 p=128, t=TLOC))
                for r in range(4):
                    nc.sync.dma_start(
                        out=v_ch[:, r, :, :],
                        in_=kvv_all[r].rearrange("(tt p c) -> p tt c",
                                                 p=128, c=C))
                # assemble v_aug: per rank, two chunk-pairs of v tiles
                va4 = v_aug[:].rearrange("p kt (h e) -> p kt h e", e=65)
                for r in range(4):
                    for half, kt0 in ((0, 2 * r), (1, 14 - 2 * r)):
                        src = v_ch[:, r, 2 * half:2 * half + 2, :].rearrange(
                            "p l (h e) -> p l h e", e=64)
                        nc.vector.tensor_copy(
                            va4[:, kt0:kt0 + 2, :, 0:64], src)
                nc.vector.memset(va4[:, :, :, 64:65], 1.0)

                def k_ap_of(kt, h):
                    ck = kt // 2
                    r = _rank_of_chunk(ck)
                    loc = _loc_of_chunk(ck) + (kt % 2) * 128
                    return k_ch[(h % 2) * 64:(h % 2) * 64 + 64, r, h // 2,
                                loc:loc + 128]

                def finalize_head(h, pav):
                    d_sb = small.tile([1, TLOC], F32, tag="dsb", name="dsb")
                    nc.vector.tensor_copy(d_sb, pav[64:65, :])
                    pb = bcpp.tile([64, TLOC], F32, tag="bc", name="pbc")
                    nc.tensor.matmul(pb, lhsT=ones64, rhs=d_sb,
                                     start=True, stop=True)
                    b_sb = small.tile([64, TLOC], F32, tag="bsb", name="bsb")
                    nc.vector.reciprocal(b_sb, pb)
                    nc.vector.tensor_mul(yT_all[:, h, :], pav[0:64, :], b_sb)

                def emit_av(pend):
                    h, e_sb, (g0, gn) = pend
                    pav = pavs[h]
                    off0 = _slot_off(g0)
                    for i in range(gn):
                        kt = g0 + i
                        w = _slot_w(kt)
                        so = _slot_off(kt) - off0
                        out = pav if w == 512 else pav[:, CHUNK:TLOC]
                        nc.tensor.matmul(
                            out, lhsT=v_aug[:, kt, h * 65:(h + 1) * 65],
                            rhs=e_sb[:, so:so + w],
                            start=(kt == 0), stop=(kt == NKT - 1),
                            skip_group_check=True)

                pavs = {}
                pend = None
                for h in range(12):
                    q_full = qT[(h % 2) * 64:(h % 2) * 64 + 64, h // 2, :]
                    q_c1 = qT[(h % 2) * 64:(h % 2) * 64 + 64, h // 2,
                              CHUNK:TLOC]
                    pavs[h] = avpp.tile([65, TLOC], F32, tag="av",
                                        name=f"pav_{h}")
                    for (g0, gn) in GROUPS:
                        pe = epp.tile([128, 1024], F32, tag="e")
                        off0 = _slot_off(g0)
                        for i in range(gn):
                            kt = g0 + i
                            w = _slot_w(kt)
                            so = _slot_off(kt) - off0
                            nc.tensor.matmul(
                                pe[:, so:so + w], lhsT=k_ap_of(kt, h),
                                rhs=(q_full if w == 512 else q_c1),
                                start=True, stop=True)
                        e_sb = esbp.tile([128, 1024], BF16, tag="esb")
                        nc.scalar.activation(
                            out=e_sb, in_=pe,
                            func=mybir.ActivationFunctionType.Exp)
                        nc.vector.tensor_mul(
                            e_sb, e_sb, masks_sb[:, off0:off0 + 1024])
                        if pend is not None:
                            emit_av(pend)
                            if pend[2][0] + pend[2][1] == NKT:
                                finalize_head(pend[0], pavs[pend[0]])
                                del pavs[pend[0]]
                        pend = (h, e_sb, (g0, gn))
                if pend is not None:
                    emit_av(pend)
                    finalize_head(pend[0], pavs[pend[0]])
                    del pavs[pend[0]]
                    pend = None

            # ---------------- proj + residual + LN2 ----------------
            with tc.tile_pool(name="pp", bufs=2, space="PSUM") as ppp, \
                 tc.tile_pool(name="ln2", bufs=3) as ln2p, \
                 tc.tile_pool(name="tp2", bufs=2, space="PSUM") as tpp2:

                xn2s = []
                for t in range(4):
                    pp = ppp.tile([128, C], F32, tag="pp")
                    for h in range(12):
                        y_ap = yT_all[:, h, t * 128:(t + 1) * 128]
                        nc.tensor.matmul(pp[:, 0:512], lhsT=y_ap,
                                         rhs=wp_sb[:, h, 0:512],
                                         start=(h == 0), stop=(h == 11))
                        nc.tensor.matmul(pp[:, 512:768], lhsT=y_ap,
                                         rhs=wp_sb[:, h, 512:768],
                                         start=(h == 0), stop=(h == 11))
                    nc.vector.tensor_add(x_sb[:, t, :], x_sb[:, t, :], pp)
                    if add_proj_bias:
                        nc.vector.tensor_add(x_sb[:, t, :], x_sb[:, t, :],
                                             bout_sb[:, 0, :])
                    xn2 = ln2p.tile([128, C], BF16, tag="xn2", name="xn2")
                    layernorm_to(ln2p, x_sb[:, t, :], xn2, "2")
                    xn2s.append(xn2)
                for t in range(4):
                    for ct in range(6):
                        pt = tpp2.tile([128, 128], BF16, tag="tp2")
                        nc.tensor.transpose(
                            pt, xn2s[t][:, ct * 128:(ct + 1) * 128], ident)
                        nc.vector.tensor_copy(
                            hT[:, ct, t * 128:(t + 1) * 128], pt)

            # ---------------- MLP ----------------
            with tc.tile_pool(name="mlp", bufs=1) as mlpp, \
                 tc.tile_pool(name="wfc", bufs=6) as wfcp, \
                 tc.tile_pool(name="wfc2", bufs=6) as wfc2p, \
                 tc.tile_pool(name="osb", bufs=3) as osbp:

                gT = mlpp.tile([128, 24, TLOC], BF16)
                wfc_t = wfc_ext.ap().rearrange("(k p) n -> p k n", p=128)
                with tc.tile_pool(name="fcp", bufs=2, space="PSUM") as fcpp:
                    for m in range(24):
                        wt = wfcp.tile([128, 6, 128], BF16, tag="wfc")
                        nc.sync.dma_start(
                            out=wt, in_=wfc_t[:, :, m * 128:(m + 1) * 128])
                        pf = fcpp.tile([128, TLOC], F32, tag="fc")
                        for k in range(6):
                            nc.tensor.matmul(pf, lhsT=wt[:, k, :],
                                             rhs=hT[:, k, :],
                                             start=(k == 0), stop=(k == 5))
                        nc.scalar.activation(
                            out=gT[:, m, :], in_=pf,
                            func=mybir.ActivationFunctionType.Gelu_apprx_tanh,
                            bias=bfc_sb[:, m:m + 1])

                wfc2_t = wfc2_ext.ap().rearrange("(k p) n -> k p n", p=128)
                with tc.tile_pool(name="f2p", bufs=1, space="PSUM") as f2pp:
                    pf2s = [f2pp.tile([128, C], F32, tag=f"f2_{t}",
                                      name=f"pf2_{t}")
                            for t in range(4)]
                    for k in range(24):
                        wt2 = wfc2p.tile([128, C], BF16, tag="wfc2")
                        nc.sync.dma_start(out=wt2, in_=wfc2_t[k])
                        for t in range(4):
                            nc.tensor.matmul(
                                pf2s[t][:, 0:512],
                                lhsT=gT[:, k, t * 128:(t + 1) * 128],
                                rhs=wt2[:, 0:512],
                                start=(k == 0), stop=(k == 23))
                            nc.tensor.matmul(
                                pf2s[t][:, 512:768],
                                lhsT=gT[:, k, t * 128:(t + 1) * 128],
                                rhs=wt2[:, 512:768],
                                start=(k == 0), stop=(k == 23))
                    for t in range(4):
                        o_sb = osbp.tile([128, C], F32, tag="osb", name="osb")
                        nc.vector.tensor_add(o_sb, x_sb[:, t, :], pf2s[t])
                        if add_fc2_bias:
                            nc.vector.tensor_add(o_sb, o_sb, bout_sb[:, 1, :])
                        nc.sync.dma_start(
                            out=out_ext[t * 128:(t + 1) * 128, :], in_=o_sb)

    nc.compile()
    return nc


def _preprocess(inputs):
    f = lambda k: np.asarray(inputs[k], np.float32)
    x = f("x"); w_attn = f("w_attn"); b_attn = f("b_attn")
    w_proj = f("w_proj"); b_proj = f("b_proj")
    w_fc = f("w_fc"); b_fc = f("b_fc"); w_fc2 = f("w_fc2"); b_fc2 = f("b_fc2")
    ln1_g = f("ln1_g"); ln1_b = f("ln1_b"); ln2_g = f("ln2_g"); ln2_b = f("ln2_b")

    w_attn_eff = ln1_g[:, None] * w_attn
    b_attn_eff = b_attn + ln1_b @ w_attn
    s = 1.0 / np.sqrt(HD)
    w_q = w_attn_eff[:, 0:C] * s
    w_k = w_attn_eff[:, C:2 * C]
    w_v = w_attn_eff[:, 2 * C:3 * C]
    b_q = b_attn_eff[0:C] * s
    b_k = b_attn_eff[C:2 * C]
    b_v = b_attn_eff[2 * C:3 * C]
    b_proj_eff = b_proj + b_v @ w_proj
    w_fc_eff = ln2_g[:, None] * w_fc
    b_fc_eff = b_fc + ln2_b @ w_fc

    wq16 = np.ascontiguousarray(w_q.astype(BF))
    wk16 = np.ascontiguousarray(w_k.astype(BF))
    wv16 = np.ascontiguousarray(w_v.astype(BF))
    wp16 = np.ascontiguousarray(w_proj.reshape(12, 64, C).astype(BF))
    wfc16 = np.ascontiguousarray(w_fc_eff.astype(BF))
    wfc216 = np.ascontiguousarray(w_fc2.astype(BF))

    bqk = np.stack([b_q, b_k]).astype(np.float32)
    bout = np.stack([b_proj_eff, b_fc2]).astype(np.float32)

    flags = (bool(np.any(bqk != 0)), bool(np.any(b_proj_eff != 0)),
             bool(np.any(b_fc2 != 0)))

    # mask slab [128, 6144] per core group j; kt<8 slots cover both q-chunks
    kpos = np.arange(128)
    qpos = np.arange(CHUNK)
    masks = np.zeros((4, 128, MASK_W), np.float32)
    for j in range(4):
        for kt in range(NKT):
            gk = kt * 128 + kpos[:, None]
            off = _slot_off(kt)
            if kt < 8:
                gq0 = j * CHUNK + qpos[None, :]
                gq1 = (7 - j) * CHUNK + qpos[None, :]
                masks[j, :, off:off + 256] = (gq0 >= gk)
                masks[j, :, off + 256:off + 512] = (gq1 >= gk)
            else:
                gq1 = (7 - j) * CHUNK + qpos[None, :]
                masks[j, :, off:off + 256] = (gq1 >= gk)
    masks16 = masks.astype(BF)

    in_maps = []
    for c in range(NCORES):
        b, j = c // 4, c % 4
        x_loc = np.concatenate(
            [x[b, j * CHUNK:(j + 1) * CHUNK],
             x[b, (7 - j) * CHUNK:(8 - j) * CHUNK]]).astype(np.float32)
        in_maps.append({
            "x": np.ascontiguousarray(x_loc),
            "wq": wq16, "wk": wk16, "wv": wv16, "wp": wp16,
            "wfc": wfc16, "wfc2": wfc216,
            "masks": np.ascontiguousarray(masks16[j]),
            "bqk": bqk, "bfc": b_fc_eff.astype(np.float32), "bout": bout,
        })
    return in_maps, flags


def kernel(**inputs):
    global LAST_EXEC_NS, LAST_RESULTS
    in_maps, flags = _preprocess(inputs)
    if flags not in _CACHE:
        _CACHE[flags] = _build(*flags)
    nc = _CACHE[flags]
    trace = bool(os.environ.get("BASS_KERNEL_TRACE"))
    res = run_bass_kernel_spmd(nc, in_maps, core_ids=list(range(NCORES)),
                               trace=trace)
    LAST_EXEC_NS = res.exec_time_ns
    LAST_RESULTS = res
    out = np.empty((B, T, C), np.float32)
    for c in range(NCORES):
        b, j = c // 4, c % 4
        o = res.results[c]["out"]
        out[b, j * CHUNK:(j + 1) * CHUNK] = o[0:CHUNK]
        out[b, (7 - j) * CHUNK:(8 - j) * CHUNK] = o[CHUNK:TLOC]
    return out


# revision 16
# speedup vs baseline: 1.1050x; 1.0249x over previous
"""GPT-2 style transformer block on 8 TRN2 NeuronCores.

Sharding: token-data-parallel. Each batch's 2048 tokens are split into 8
chunks of 256; core c owns batch c//4 and chunks {j, 7-j} (j = c%4) so
causal attention work is balanced. QKV/proj/MLP/LN are purely local; the
only collectives are two AllGathers (k^T, then v) within each 4-core
batch group. Causality is enforced with per-core 0/1 mask tensors so all
cores run one identical SPMD graph (uniform loop bounds; masks zero the
beyond-causal tiles, which also makes the per-core graphs j-independent).

Matmuls run in bf16 (f32 PSUM accumulation); LN/softmax/residuals in f32.
LN affine params are folded into the following matmul weights host-side;
the attention 1/sqrt(hd) scale is folded into w_q; the v-bias is folded
into the proj bias via the softmax-rows-sum-to-one identity. Softmax is
computed without max-subtraction (scores are O(1) here, exp cannot
overflow in f32) as exp(s) normalized by a denominator obtained for free
as an extra ones-column in the av matmul. Both local q-chunks share one
[65, 512] av accumulator per head; key tiles 0..7 are scored against all
512 local queries in one matmul, tiles 8..15 only against q-chunk 1.
"""

import os
import sys

sys.path.insert(0, "/opt/trn_rl_repo")

import numpy as np
import ml_dtypes

import concourse.bass as bass
import concourse.tile as tile
from concourse import bacc, mybir
from concourse.bass_utils import run_bass_kernel_spmd
from concourse.masks import make_identity

F32 = mybir.dt.float32
BF16 = mybir.dt.bfloat16
BF = ml_dtypes.bfloat16

B, T, C, H, HD = 2, 2048, 768, 12, 64
EPS = 1e-5
NCORES = 8
CHUNK = 256            # global chunk size (tokens)
TLOC = 512             # local tokens per core (2 chunks)
NKT = T // 128         # 16 key tiles per batch
CC = T // CHUNK        # 8 chunks per batch

# e-slot layout: kt<8 -> 512 wide (both q-chunks), kt>=8 -> 256 (q-chunk 1)
def _slot_off(kt):
    return kt * 512 if kt < 8 else 4096 + (kt - 8) * 256


def _slot_w(kt):
    return 512 if kt < 8 else 256


MASK_W = 8 * 512 + 8 * 256   # 6144
# exp groups: contiguous 1024-col spans of the slot layout
GROUPS = [(0, 2), (2, 2), (4, 2), (6, 2), (8, 4), (12, 4)]

KT_ELEMS = 6 * 128 * TLOC          # k^T bounce: [6 ct][128 p][512 t]
V_ELEMS = 4 * 128 * C              # v bounce:   [4 tt][128 p][768 c]

LAST_EXEC_NS = None
LAST_RESULTS = None
_CACHE = {}


def _rank_of_chunk(ck):
    return ck if ck < 4 else 7 - ck


def _loc_of_chunk(ck):
    return 0 if ck < 4 else CHUNK


def _build(add_qk_bias, add_proj_bias, add_fc2_bias):
    nc = bacc.Bacc("TRN2", target_bir_lowering=False, debug=False,
                   num_devices=NCORES)

    x_ext = nc.dram_tensor("x", [TLOC, C], F32, kind="ExternalInput")
    wq_ext = nc.dram_tensor("wq", [C, C], BF16, kind="ExternalInput")
    wk_ext = nc.dram_tensor("wk", [C, C], BF16, kind="ExternalInput")
    wv_ext = nc.dram_tensor("wv", [C, C], BF16, kind="ExternalInput")
    wp_ext = nc.dram_tensor("wp", [12, 64, C], BF16, kind="ExternalInput")
    wfc_ext = nc.dram_tensor("wfc", [C, 4 * C], BF16, kind="ExternalInput")
    wfc2_ext = nc.dram_tensor("wfc2", [4 * C, C], BF16, kind="ExternalInput")
    masks_ext = nc.dram_tensor("masks", [128, MASK_W], BF16,
                               kind="ExternalInput")
    bqk_ext = nc.dram_tensor("bqk", [2, C], F32, kind="ExternalInput")
    bfc_ext = nc.dram_tensor("bfc", [4 * C], F32, kind="ExternalInput")
    bout_ext = nc.dram_tensor("bout", [2, C], F32, kind="ExternalInput")
    out_ext = nc.dram_tensor("out", [TLOC, C], F32, kind="ExternalOutput")

    with tile.TileContext(nc) as tc:
        with tc.tile_pool(name="dram", bufs=1, space="DRAM") as dram, \
             tc.tile_pool(name="singles", bufs=1) as singles, \
             tc.tile_pool(name="persist", bufs=1) as persist, \
             tc.tile_pool(name="small", bufs=3) as small:

            kvk_in = dram.tile([KT_ELEMS], BF16)
            kvk_all = dram.tile([4, KT_ELEMS], BF16)
            kvv_in = dram.tile([V_ELEMS], BF16)
            kvv_all = dram.tile([4, V_ELEMS], BF16)

            ident = singles.tile([128, 128], BF16)
            make_identity(nc, ident)
            eps_sb = singles.tile([128, 1], F32)
            nc.vector.memset(eps_sb, EPS)
            ones64 = singles.tile([1, 64], F32)
            nc.vector.memset(ones64, 1.0)

            wq_sb = persist.tile([128, 6, C], BF16)
            wk_sb = persist.tile([128, 6, C], BF16)
            wv_sb = persist.tile([128, 6, C], BF16)
            wp_sb = persist.tile([64, 12, C], BF16)
            for sb, ext in ((wq_sb, wq_ext), (wk_sb, wk_ext),
                            (wv_sb, wv_ext)):
                nc.sync.dma_start(
                    out=sb, in_=ext.ap().rearrange("(ct p) c -> p ct c", p=128))
            nc.sync.dma_start(
                out=wp_sb, in_=wp_ext.ap().rearrange("h p c -> p h c"))

            bqk_sb = singles.tile([128, 2, 6], F32)
            if add_qk_bias:
                nc.sync.dma_start(
                    out=bqk_sb,
                    in_=bqk_ext.ap().rearrange("b (m p) -> p b m", p=128))
            bfc_sb = singles.tile([128, 24], F32)
            nc.sync.dma_start(
                out=bfc_sb, in_=bfc_ext.ap().rearrange("(m p) -> p m", p=128))
            bout_sb = singles.tile([128, 2, C], F32)
            if add_proj_bias or add_fc2_bias:
                bc = bout_ext.ap()
                nc.sync.dma_start(
                    out=bout_sb,
                    in_=bass.AP(tensor=bc.tensor, offset=bc.offset,
                                ap=[[0, 128], bc.ap[0], bc.ap[1]]))

            masks_sb = persist.tile([128, MASK_W], BF16)
            nc.sync.dma_start(out=masks_sb, in_=masks_ext.ap())

            x_sb = persist.tile([128, 4, C], F32)     # local x, becomes xmid
            hT = persist.tile([128, 6, TLOC], BF16)   # h^T, reused for h2^T
            qT = persist.tile([128, 6, TLOC], BF16)
            yT_all = persist.tile([64, 12, TLOC], BF16)

            def layernorm_to(pool, xt, dst, tagsuf):
                stats = pool.tile([128, 3, 6], F32, tag="st" + tagsuf,
                                  name="st" + tagsuf)
                for sg in range(3):
                    nc.vector.bn_stats(out=stats[:, sg, :],
                                       in_=xt[:, sg * 256:(sg + 1) * 256])
                mv = pool.tile([128, 2], F32, tag="mv" + tagsuf,
                               name="mv" + tagsuf)
                nc.vector.bn_aggr(out=mv, in_=stats)
                nc.scalar.activation(out=mv[:, 1:2], in_=mv[:, 1:2],
                                     func=mybir.ActivationFunctionType.Sqrt,
                                     bias=eps_sb)
                nc.vector.reciprocal(out=mv[:, 1:2], in_=mv[:, 1:2])
                nc.vector.tensor_scalar(out=dst, in0=xt,
                                        scalar1=mv[:, 0:1], scalar2=mv[:, 1:2],
                                        op0=mybir.AluOpType.subtract,
                                        op1=mybir.AluOpType.mult)

            # ---------------- LN1 + transpose + QKV + AGs ----------------
            with tc.tile_pool(name="ln", bufs=3) as lnp, \
                 tc.tile_pool(name="tp", bufs=2, space="PSUM") as tpp, \
                 tc.tile_pool(name="qkp", bufs=2, space="PSUM") as qkp, \
                 tc.tile_pool(name="vp", bufs=2, space="PSUM") as vpp, \
                 tc.tile_pool(name="vsb", bufs=1) as vsbp:

                kT = vsbp.tile([128, 6, TLOC], BF16)
                for t in range(4):
                    nc.sync.dma_start(out=x_sb[:, t, :],
                                      in_=x_ext[t * 128:(t + 1) * 128, :])
                    xn = lnp.tile([128, C], BF16, tag="xn")
                    layernorm_to(lnp, x_sb[:, t, :], xn, "1")
                    for ct in range(6):
                        pt = tpp.tile([128, 128], BF16, tag="tp")
                        nc.tensor.transpose(
                            pt, xn[:, ct * 128:(ct + 1) * 128], ident)
                        nc.vector.tensor_copy(
                            hT[:, ct, t * 128:(t + 1) * 128], pt)

                # k^T first: it feeds the first collective
                for m in range(6):
                    ps = qkp.tile([128, TLOC], F32, tag="qk")
                    for k in range(6):
                        nc.tensor.matmul(
                            ps, lhsT=wk_sb[:, k, m * 128:(m + 1) * 128],
                            rhs=hT[:, k, :], start=(k == 0), stop=(k == 5))
                    if add_qk_bias:
                        nc.vector.tensor_scalar_add(
                            out=kT[:, m, :], in0=ps,
                            scalar1=bqk_sb[:, 1, m:m + 1])
                    else:
                        nc.vector.tensor_copy(kT[:, m, :], ps)
                nc.sync.dma_start(
                    out=kvk_in[:].rearrange("(ct p t) -> p ct t", p=128, t=TLOC),
                    in_=kT)
                nc.gpsimd.collective_compute(
                    "AllGather", mybir.AluOpType.bypass,
                    replica_groups=[[0, 1, 2, 3], [4, 5, 6, 7]],
                    ins=[kvk_in[:].opt()], outs=[kvk_all[:].opt()])

                v_sb = vsbp.tile([128, 4, C], BF16)
                for tt in range(4):
                    pv = vpp.tile([128, C], F32, tag="v")
                    for k in range(6):
                        nc.tensor.matmul(
                            pv[:, 0:512],
                            lhsT=hT[:, k, tt * 128:(tt + 1) * 128],
                            rhs=wv_sb[:, k, 0:512],
                            start=(k == 0), stop=(k == 5))
                        nc.tensor.matmul(
                            pv[:, 512:768],
                            lhsT=hT[:, k, tt * 128:(tt + 1) * 128],
                            rhs=wv_sb[:, k, 512:768],
                            start=(k == 0), stop=(k == 5))
                    nc.vector.tensor_copy(v_sb[:, tt, :], pv)
                nc.sync.dma_start(
                    out=kvv_in[:].rearrange("(tt p c) -> p tt c", p=128, c=C),
                    in_=v_sb)
                nc.gpsimd.collective_compute(
                    "AllGather", mybir.AluOpType.bypass,
                    replica_groups=[[0, 1, 2, 3], [4, 5, 6, 7]],
                    ins=[kvv_in[:].opt()], outs=[kvv_all[:].opt()])

                for m in range(6):
                    ps = qkp.tile([128, TLOC], F32, tag="qk")
                    for k in range(6):
                        nc.tensor.matmul(
                            ps, lhsT=wq_sb[:, k, m * 128:(m + 1) * 128],
                            rhs=hT[:, k, :], start=(k == 0), stop=(k == 5))
                    if add_qk_bias:
                        nc.vector.tensor_scalar_add(
                            out=qT[:, m, :], in0=ps,
                            scalar1=bqk_sb[:, 0, m:m + 1])
                    else:
                        nc.vector.tensor_copy(qT[:, m, :], ps)

            # ---------------- attention ----------------
            with tc.tile_pool(name="kch", bufs=1) as kchp, \
                 tc.tile_pool(name="vch", bufs=1) as vchp, \
                 tc.tile_pool(name="vaug", bufs=1) as vaugp, \
                 tc.tile_pool(name="ep", bufs=2, space="PSUM") as epp, \
                 tc.tile_pool(name="avp", bufs=2, space="PSUM") as avpp, \
                 tc.tile_pool(name="bcp", bufs=1, space="PSUM") as bcpp, \
                 tc.tile_pool(name="esb", bufs=4) as esbp:

                # gathered k: [r][6 ct][128][512]; v: [r][4 lt][128][768]
                k_ch = kchp.tile([128, 4, 6, TLOC], BF16)
                v_ch = vchp.tile([128, 4, 4, C], BF16)
                v_aug = vaugp.tile([128, NKT, 12 * 65], BF16)

                for r in range(4):
                    nc.sync.dma_start(
                        out=k_ch[:, r, :, :],
                        in_=kvk_all[r].rearrange("(ct p t) -> p ct t",
                                                 p=128, t=TLOC))
                for r in range(4):
                    nc.sync.dma_start(
                        out=v_ch[:, r, :, :],
                        in_=kvv_all[r].rearrange("(tt p c) -> p tt c",
                                                 p=128, c=C))
                # assemble v_aug: per rank, two chunk-pairs of v tiles
                va4 = v_aug[:].rearrange("p kt (h e) -> p kt h e", e=65)
                nc.vector.memset(va4[:, :, :, 64:65], 1.0)
                for r in range(4):
                    for half, kt0 in ((0, 2 * r), (1, 14 - 2 * r)):
                        vsrc = v_ch[:, r, 2 * half:2 * half + 2, :].rearrange(
                            "p l (h e) -> p l h e", e=64)
                        nc.vector.tensor_copy(
                            va4[:, kt0:kt0 + 2, :, 0:64], vsrc)

                def k_ap_of(kt, h):
                    ck = kt // 2
                    r = _rank_of_chunk(ck)
                    loc = _loc_of_chunk(ck) + (kt % 2) * 128
                    return k_ch[(h % 2) * 64:(h % 2) * 64 + 64, r, h // 2,
                                loc:loc + 128]

                def finalize_head(h, pav):
                    d_sb = small.tile([1, TLOC], F32, tag="dsb", name="dsb")
                    nc.vector.tensor_copy(d_sb, pav[64:65, :])
                    pb = bcpp.tile([64, TLOC], F32, tag="bc", name="pbc")
                    nc.tensor.matmul(pb, lhsT=ones64, rhs=d_sb,
                                     start=True, stop=True)
                    b_sb = small.tile([64, TLOC], F32, tag="bsb", name="bsb")
                    nc.vector.reciprocal_approx_fast(out=b_sb, in_=pb)
                    nc.vector.tensor_mul(yT_all[:, h, :], pav[0:64, :], b_sb)

                def emit_av(pend):
                    h, e_sb, (g0, gn) = pend
                    pav = pavs[h]
                    off0 = _slot_off(g0)
                    for i in range(gn):
                        kt = g0 + i
                        w = _slot_w(kt)
                        so = _slot_off(kt) - off0
                        out = pav if w == 512 else pav[:, CHUNK:TLOC]
                        nc.tensor.matmul(
                            out, lhsT=v_aug[:, kt, h * 65:(h + 1) * 65],
                            rhs=e_sb[:, so:so + w],
                            start=(kt == 0), stop=(kt == NKT - 1),
                            skip_group_check=True)

                pavs = {}
                pends = []
                for h in range(12):
                    q_full = qT[(h % 2) * 64:(h % 2) * 64 + 64, h // 2, :]
                    q_c1 = qT[(h % 2) * 64:(h % 2) * 64 + 64, h // 2,
                              CHUNK:TLOC]
                    pavs[h] = avpp.tile([65, TLOC], F32, tag="av",
                                        name=f"pav_{h}")
                    for (g0, gn) in GROUPS:
                        pe = epp.tile([128, 1024], F32, tag="e")
                        off0 = _slot_off(g0)
                        for i in range(gn):
                            kt = g0 + i
                            w = _slot_w(kt)
                            so = _slot_off(kt) - off0
                            nc.tensor.matmul(
                                pe[:, so:so + w], lhsT=k_ap_of(kt, h),
                                rhs=(q_full if w == 512 else q_c1),
                                start=True, stop=True)
                        e_sb = esbp.tile([128, 1024], BF16, tag="esb")
                        nc.scalar.activation(
                            out=e_sb, in_=pe,
                            func=mybir.ActivationFunctionType.Exp)
                        nc.vector.tensor_mul(
                            e_sb, e_sb, masks_sb[:, off0:off0 + 1024])
                        pends.append((h, e_sb, (g0, gn)))
                        if len(pends) > 2:
                            pend = pends.pop(0)
                            emit_av(pend)
                            if pend[2][0] + pend[2][1] == NKT:
                                finalize_head(pend[0], pavs[pend[0]])
                                del pavs[pend[0]]
                for pend in pends:
                    emit_av(pend)
                    if pend[2][0] + pend[2][1] == NKT:
                        finalize_head(pend[0], pavs[pend[0]])
                        del pavs[pend[0]]
                pends = []

            # ---------------- proj + residual + LN2 ----------------
            with tc.tile_pool(name="pp", bufs=2, space="PSUM") as ppp, \
                 tc.tile_pool(name="ln2", bufs=3) as ln2p, \
                 tc.tile_pool(name="tp2", bufs=2, space="PSUM") as tpp2:

                xn2s = []
                for t in range(4):
                    pp = ppp.tile([128, C], F32, tag="pp")
                    for h in range(12):
                        y_ap = yT_all[:, h, t * 128:(t + 1) * 128]
                        nc.tensor.matmul(pp[:, 0:512], lhsT=y_ap,
                                         rhs=wp_sb[:, h, 0:512],
                                         start=(h == 0), stop=(h == 11))
                        nc.tensor.matmul(pp[:, 512:768], lhsT=y_ap,
                                         rhs=wp_sb[:, h, 512:768],
                                         start=(h == 0), stop=(h == 11))
                    nc.vector.tensor_add(x_sb[:, t, :], x_sb[:, t, :], pp)
                    if add_proj_bias:
                        nc.vector.tensor_add(x_sb[:, t, :], x_sb[:, t, :],
                                             bout_sb[:, 0, :])
                    xn2 = ln2p.tile([128, C], BF16, tag="xn2", name="xn2")
                    layernorm_to(ln2p, x_sb[:, t, :], xn2, "2")
                    xn2s.append(xn2)
                for t in range(4):
                    for ct in range(6):
                        pt = tpp2.tile([128, 128], BF16, tag="tp2")
                        nc.tensor.transpose(
                            pt, xn2s[t][:, ct * 128:(ct + 1) * 128], ident)
                        nc.vector.tensor_copy(
                            hT[:, ct, t * 128:(t + 1) * 128], pt)

            # ---------------- MLP ----------------
            with tc.tile_pool(name="mlp", bufs=1) as mlpp, \
                 tc.tile_pool(name="wfc", bufs=6) as wfcp, \
                 tc.tile_pool(name="wfc2", bufs=6) as wfc2p, \
                 tc.tile_pool(name="osb", bufs=3) as osbp:

                gT = mlpp.tile([128, 24, TLOC], BF16)
                wfc_t = wfc_ext.ap().rearrange("(k p) n -> p k n", p=128)
                with tc.tile_pool(name="fcp", bufs=2, space="PSUM") as fcpp:
                    for m in range(24):
                        wt = wfcp.tile([128, 6, 128], BF16, tag="wfc")
                        nc.sync.dma_start(
                            out=wt, in_=wfc_t[:, :, m * 128:(m + 1) * 128])
                        pf = fcpp.tile([128, TLOC], F32, tag="fc")
                        for k in range(6):
                            nc.tensor.matmul(pf, lhsT=wt[:, k, :],
                                             rhs=hT[:, k, :],
                                             start=(k == 0), stop=(k == 5))
                        nc.scalar.activation(
                            out=gT[:, m, :], in_=pf,
                            func=mybir.ActivationFunctionType.Gelu_apprx_tanh,
                            bias=bfc_sb[:, m:m + 1])

                wfc2_t = wfc2_ext.ap().rearrange("(k p) n -> k p n", p=128)
                with tc.tile_pool(name="f2p", bufs=1, space="PSUM") as f2pp:
                    pf2s = [f2pp.tile([128, C], F32, tag=f"f2_{t}",
                                      name=f"pf2_{t}")
                            for t in range(4)]
                    for k in range(24):
                        wt2 = wfc2p.tile([128, C], BF16, tag="wfc2")
                        nc.sync.dma_start(out=wt2, in_=wfc2_t[k])
                        for t in range(4):
                            nc.tensor.matmul(
                                pf2s[t][:, 0:512],
                                lhsT=gT[:, k, t * 128:(t + 1) * 128],
                                rhs=wt2[:, 0:512],
                                start=(k == 0), stop=(k == 23))
                            nc.tensor.matmul(
                                pf2s[t][:, 512:768],
                                lhsT=gT[:, k, t * 128:(t + 1) * 128],
                                rhs=wt2[:, 512:768],
                                start=(k == 0), stop=(k == 23))
                    for t in range(4):
                        o_sb = osbp.tile([128, C], F32, tag="osb", name="osb")
                        nc.vector.tensor_add(o_sb, x_sb[:, t, :], pf2s[t])
                        if add_fc2_bias:
                            nc.vector.tensor_add(o_sb, o_sb, bout_sb[:, 1, :])
                        nc.sync.dma_start(
                            out=out_ext[t * 128:(t + 1) * 128, :], in_=o_sb)

    nc.compile()
    return nc


def _preprocess(inputs):
    f = lambda k: np.asarray(inputs[k], np.float32)
    x = f("x"); w_attn = f("w_attn"); b_attn = f("b_attn")
    w_proj = f("w_proj"); b_proj = f("b_proj")
    w_fc = f("w_fc"); b_fc = f("b_fc"); w_fc2 = f("w_fc2"); b_fc2 = f("b_fc2")
    ln1_g = f("ln1_g"); ln1_b = f("ln1_b"); ln2_g = f("ln2_g"); ln2_b = f("ln2_b")

    w_attn_eff = ln1_g[:, None] * w_attn
    b_attn_eff = b_attn + ln1_b @ w_attn
    s = 1.0 / np.sqrt(HD)
    w_q = w_attn_eff[:, 0:C] * s
    w_k = w_attn_eff[:, C:2 * C]
    w_v = w_attn_eff[:, 2 * C:3 * C]
    b_q = b_attn_eff[0:C] * s
    b_k = b_attn_eff[C:2 * C]
    b_v = b_attn_eff[2 * C:3 * C]
    b_proj_eff = b_proj + b_v @ w_proj
    w_fc_eff = ln2_g[:, None] * w_fc
    b_fc_eff = b_fc + ln2_b @ w_fc

    wq16 = np.ascontiguousarray(w_q.astype(BF))
    wk16 = np.ascontiguousarray(w_k.astype(BF))
    wv16 = np.ascontiguousarray(w_v.astype(BF))
    wp16 = np.ascontiguousarray(w_proj.reshape(12, 64, C).astype(BF))
    wfc16 = np.ascontiguousarray(w_fc_eff.astype(BF))
    wfc216 = np.ascontiguousarray(w_fc2.astype(BF))

    bqk = np.stack([b_q, b_k]).astype(np.float32)
    bout = np.stack([b_proj_eff, b_fc2]).astype(np.float32)

    flags = (bool(np.any(bqk != 0)), bool(np.any(b_proj_eff != 0)),
             bool(np.any(b_fc2 != 0)))

    # mask slab [128, 6144] per core group j; kt<8 slots cover both q-chunks
    kpos = np.arange(128)
    qpos = np.arange(CHUNK)
    masks = np.zeros((4, 128, MASK_W), np.float32)
    for j in range(4):
        for kt in range(NKT):
            gk = kt * 128 + kpos[:, None]
            off = _slot_off(kt)
            if kt < 8:
                gq0 = j * CHUNK + qpos[None, :]
                gq1 = (7 - j) * CHUNK + qpos[None, :]
                masks[j, :, off:off + 256] = (gq0 >= gk)
                masks[j, :, off + 256:off + 512] = (gq1 >= gk)
            else:
                gq1 = (7 - j) * CHUNK + qpos[None, :]
                masks[j, :, off:off + 256] = (gq1 >= gk)
    masks16 = masks.astype(BF)

    in_maps = []
    for c in range(NCORES):
        b, j = c // 4, c % 4
        x_loc = np.concatenate(
            [x[b, j * CHUNK:(j + 1) * CHUNK],
             x[b, (7 - j) * CHUNK:(8 - j) * CHUNK]]).astype(np.float32)
        in_maps.append({
            "x": np.ascontiguousarray(x_loc),
            "wq": wq16, "wk": wk16, "wv": wv16, "wp": wp16,
            "wfc": wfc16, "wfc2": wfc216,
            "masks": np.ascontiguousarray(masks16[j]),
            "bqk": bqk, "bfc": b_fc_eff.astype(np.float32), "bout": bout,
        })
    return in_maps, flags


def kernel(**inputs):
    global LAST_EXEC_NS, LAST_RESULTS
    in_maps, flags = _preprocess(inputs)
    if flags not in _CACHE:
        _CACHE[flags] = _build(*flags)
    nc = _CACHE[flags]
    trace = bool(os.environ.get("BASS_KERNEL_TRACE"))
    res = run_bass_kernel_spmd(nc, in_maps, core_ids=list(range(NCORES)),
                               trace=trace)
    LAST_EXEC_NS = res.exec_time_ns
    LAST_RESULTS = res
    out = np.empty((B, T, C), np.float32)
    for c in range(NCORES):
        b, j = c // 4, c % 4
        o = res.results[c]["out"]
        out[b, j * CHUNK:(j + 1) * CHUNK] = o[0:CHUNK]
        out[b, (7 - j) * CHUNK:(8 - j) * CHUNK] = o[CHUNK:TLOC]
    return out


# revision 17
# speedup vs baseline: 1.1174x; 1.0112x over previous
"""GPT-2 style transformer block on 8 TRN2 NeuronCores.

Sharding: token-data-parallel. Each batch's 2048 tokens are split into 8
chunks of 256; core c owns batch c//4 and chunks {j, 7-j} (j = c%4) so
causal attention work is balanced. QKV/proj/MLP/LN are purely local; the
only collectives are two AllGathers (k^T, then v) within each 4-core
batch group. Causality is enforced with per-core 0/1 mask tensors so all
cores run one identical SPMD graph (uniform loop bounds; masks zero the
beyond-causal tiles, which also makes the per-core graphs j-independent).

Matmuls run in bf16 (f32 PSUM accumulation); LN/softmax/residuals in f32.
LN affine params are folded into the following matmul weights host-side;
the attention 1/sqrt(hd) scale is folded into w_q; the v-bias is folded
into the proj bias via the softmax-rows-sum-to-one identity. Softmax is
computed without max-subtraction (scores are O(1) here, exp cannot
overflow in f32) as exp(s) normalized by a denominator obtained for free
as an extra ones-column in the av matmul. Both local q-chunks share one
[65, 512] av accumulator per head; key tiles 0..7 are scored against all
512 local queries in one matmul, tiles 8..15 only against q-chunk 1.
"""

import os
import sys

sys.path.insert(0, "/opt/trn_rl_repo")

import numpy as np
import ml_dtypes

import concourse.bass as bass
import concourse.tile as tile
from concourse import bacc, mybir
from concourse.bass_utils import run_bass_kernel_spmd
from concourse.masks import make_identity

F32 = mybir.dt.float32
BF16 = mybir.dt.bfloat16
BF = ml_dtypes.bfloat16

B, T, C, H, HD = 2, 2048, 768, 12, 64
EPS = 1e-5
NCORES = 8
CHUNK = 256            # global chunk size (tokens)
TLOC = 512             # local tokens per core (2 chunks)
NKT = T // 128         # 16 key tiles per batch
CC = T // CHUNK        # 8 chunks per batch

# e-slot layout: kt<8 -> 512 wide (both q-chunks), kt>=8 -> 256 (q-chunk 1)
def _slot_off(kt):
    return kt * 512 if kt < 8 else 4096 + (kt - 8) * 256


def _slot_w(kt):
    return 512 if kt < 8 else 256


MASK_W = 8 * 512 + 8 * 256   # 6144
# exp groups: contiguous 1024-col spans of the slot layout
GROUPS = [(0, 2), (2, 2), (4, 2), (6, 2), (8, 4), (12, 4)]

KT_ELEMS = 6 * 128 * TLOC          # k^T bounce: [6 ct][128 p][512 t]
V_ELEMS = 4 * 128 * C              # v bounce:   [4 tt][128 p][768 c]

LAST_EXEC_NS = None
LAST_RESULTS = None
_CACHE = {}


def _rank_of_chunk(ck):
    return ck if ck < 4 else 7 - ck


def _loc_of_chunk(ck):
    return 0 if ck < 4 else CHUNK


def _build(add_qk_bias, add_proj_bias, add_fc2_bias):
    nc = bacc.Bacc("TRN2", target_bir_lowering=False, debug=False,
                   num_devices=NCORES)

    x_ext = nc.dram_tensor("x", [TLOC, C], F32, kind="ExternalInput")
    wq_ext = nc.dram_tensor("wq", [C, C], BF16, kind="ExternalInput")
    wk_ext = nc.dram_tensor("wk", [C, C], BF16, kind="ExternalInput")
    wv_ext = nc.dram_tensor("wv", [C, C], BF16, kind="ExternalInput")
    wp_ext = nc.dram_tensor("wp", [12, 64, C], BF16, kind="ExternalInput")
    wfc_ext = nc.dram_tensor("wfc", [C, 4 * C], BF16, kind="ExternalInput")
    wfc2_ext = nc.dram_tensor("wfc2", [4 * C, C], BF16, kind="ExternalInput")
    masks_ext = nc.dram_tensor("masks", [128, MASK_W], BF16,
                               kind="ExternalInput")
    bqk_ext = nc.dram_tensor("bqk", [2, C], F32, kind="ExternalInput")
    bfc_ext = nc.dram_tensor("bfc", [4 * C], F32, kind="ExternalInput")
    bout_ext = nc.dram_tensor("bout", [2, C], F32, kind="ExternalInput")
    out_ext = nc.dram_tensor("out", [TLOC, C], F32, kind="ExternalOutput")

    with tile.TileContext(nc) as tc:
        with tc.tile_pool(name="dram", bufs=1, space="DRAM") as dram, \
             tc.tile_pool(name="singles", bufs=1) as singles, \
             tc.tile_pool(name="persist", bufs=1) as persist, \
             tc.tile_pool(name="small", bufs=3) as small:

            kvk_in = dram.tile([KT_ELEMS], BF16)
            kvk_all = dram.tile([4, KT_ELEMS], BF16)
            kvv_in = dram.tile([V_ELEMS], BF16)
            kvv_all = dram.tile([4, V_ELEMS], BF16)

            ident = singles.tile([128, 128], BF16)
            make_identity(nc, ident)
            eps_sb = singles.tile([128, 1], F32)
            nc.vector.memset(eps_sb, EPS)
            ones64 = singles.tile([1, 64], F32)
            nc.vector.memset(ones64, 1.0)

            wq_sb = persist.tile([128, 6, C], BF16)
            wk_sb = persist.tile([128, 6, C], BF16)
            wv_sb = persist.tile([128, 6, C], BF16)
            wp_sb = persist.tile([64, 12, C], BF16)
            for sb, ext in ((wq_sb, wq_ext), (wk_sb, wk_ext),
                            (wv_sb, wv_ext)):
                nc.sync.dma_start(
                    out=sb, in_=ext.ap().rearrange("(ct p) c -> p ct c", p=128))
            nc.sync.dma_start(
                out=wp_sb, in_=wp_ext.ap().rearrange("h p c -> p h c"))

            bqk_sb = singles.tile([128, 2, 6], F32)
            if add_qk_bias:
                nc.sync.dma_start(
                    out=bqk_sb,
                    in_=bqk_ext.ap().rearrange("b (m p) -> p b m", p=128))
            bfc_sb = singles.tile([128, 24], F32)
            nc.sync.dma_start(
                out=bfc_sb, in_=bfc_ext.ap().rearrange("(m p) -> p m", p=128))
            bout_sb = singles.tile([128, 2, C], F32)
            if add_proj_bias or add_fc2_bias:
                bc = bout_ext.ap()
                nc.sync.dma_start(
                    out=bout_sb,
                    in_=bass.AP(tensor=bc.tensor, offset=bc.offset,
                                ap=[[0, 128], bc.ap[0], bc.ap[1]]))

            masks_sb = persist.tile([128, MASK_W], BF16)
            nc.sync.dma_start(out=masks_sb, in_=masks_ext.ap())

            x_sb = persist.tile([128, 4, C], F32)     # local x, becomes xmid
            hT = persist.tile([128, 6, TLOC], BF16)   # h^T, reused for h2^T
            qT = persist.tile([128, 6, TLOC], BF16)
            yT_all = persist.tile([64, 12, TLOC], BF16)

            def layernorm_to(pool, xt, dst, tagsuf):
                stats = pool.tile([128, 3, 6], F32, tag="st" + tagsuf,
                                  name="st" + tagsuf)
                for sg in range(3):
                    nc.vector.bn_stats(out=stats[:, sg, :],
                                       in_=xt[:, sg * 256:(sg + 1) * 256])
                mv = pool.tile([128, 2], F32, tag="mv" + tagsuf,
                               name="mv" + tagsuf)
                nc.vector.bn_aggr(out=mv, in_=stats)
                nc.scalar.activation(out=mv[:, 1:2], in_=mv[:, 1:2],
                                     func=mybir.ActivationFunctionType.Sqrt,
                                     bias=eps_sb)
                nc.vector.reciprocal(out=mv[:, 1:2], in_=mv[:, 1:2])
                nc.vector.tensor_scalar(out=dst, in0=xt,
                                        scalar1=mv[:, 0:1], scalar2=mv[:, 1:2],
                                        op0=mybir.AluOpType.subtract,
                                        op1=mybir.AluOpType.mult)

            # ---------------- LN1 + transpose + QKV + AGs ----------------
            with tc.tile_pool(name="ln", bufs=3) as lnp, \
                 tc.tile_pool(name="tp", bufs=2, space="PSUM") as tpp, \
                 tc.tile_pool(name="qkp", bufs=2, space="PSUM") as qkp, \
                 tc.tile_pool(name="vp", bufs=2, space="PSUM") as vpp, \
                 tc.tile_pool(name="vsb", bufs=1) as vsbp:

                kT = vsbp.tile([128, 6, TLOC], BF16)
                for t in range(4):
                    nc.sync.dma_start(out=x_sb[:, t, :],
                                      in_=x_ext[t * 128:(t + 1) * 128, :])
                    xn = lnp.tile([128, C], BF16, tag="xn")
                    layernorm_to(lnp, x_sb[:, t, :], xn, "1")
                    for ct in range(6):
                        pt = tpp.tile([128, 128], BF16, tag="tp")
                        nc.tensor.transpose(
                            pt, xn[:, ct * 128:(ct + 1) * 128], ident)
                        nc.vector.tensor_copy(
                            hT[:, ct, t * 128:(t + 1) * 128], pt)

                # k^T first: it feeds the first collective
                for m in range(6):
                    ps = qkp.tile([128, TLOC], F32, tag="qk")
                    for k in range(6):
                        nc.tensor.matmul(
                            ps, lhsT=wk_sb[:, k, m * 128:(m + 1) * 128],
                            rhs=hT[:, k, :], start=(k == 0), stop=(k == 5))
                    if add_qk_bias:
                        nc.vector.tensor_scalar_add(
                            out=kT[:, m, :], in0=ps,
                            scalar1=bqk_sb[:, 1, m:m + 1])
                    else:
                        nc.vector.tensor_copy(kT[:, m, :], ps)
                nc.sync.dma_start(
                    out=kvk_in[:].rearrange("(ct p t) -> p ct t", p=128, t=TLOC),
                    in_=kT)
                nc.gpsimd.collective_compute(
                    "AllGather", mybir.AluOpType.bypass,
                    replica_groups=[[0, 1, 2, 3], [4, 5, 6, 7]],
                    ins=[kvk_in[:].opt()], outs=[kvk_all[:].opt()])

                v_sb = vsbp.tile([128, 4, C], BF16)
                for tt in range(4):
                    pv = vpp.tile([128, C], F32, tag="v")
                    for k in range(6):
                        nc.tensor.matmul(
                            pv[:, 0:512],
                            lhsT=hT[:, k, tt * 128:(tt + 1) * 128],
                            rhs=wv_sb[:, k, 0:512],
                            start=(k == 0), stop=(k == 5))
                        nc.tensor.matmul(
                            pv[:, 512:768],
                            lhsT=hT[:, k, tt * 128:(tt + 1) * 128],
                            rhs=wv_sb[:, k, 512:768],
                            start=(k == 0), stop=(k == 5))
                    nc.vector.tensor_copy(v_sb[:, tt, :], pv)
                nc.sync.dma_start(
                    out=kvv_in[:].rearrange("(tt p c) -> p tt c", p=128, c=C),
                    in_=v_sb)
                nc.gpsimd.collective_compute(
                    "AllGather", mybir.AluOpType.bypass,
                    replica_groups=[[0, 1, 2, 3], [4, 5, 6, 7]],
                    ins=[kvv_in[:].opt()], outs=[kvv_all[:].opt()])

                for m in range(6):
                    ps = qkp.tile([128, TLOC], F32, tag="qk")
                    for k in range(6):
                        nc.tensor.matmul(
                            ps, lhsT=wq_sb[:, k, m * 128:(m + 1) * 128],
                            rhs=hT[:, k, :], start=(k == 0), stop=(k == 5))
                    if add_qk_bias:
                        nc.vector.tensor_scalar_add(
                            out=qT[:, m, :], in0=ps,
                            scalar1=bqk_sb[:, 0, m:m + 1])
                    else:
                        nc.vector.tensor_copy(qT[:, m, :], ps)

            # ---------------- attention ----------------
            with tc.tile_pool(name="kch", bufs=1) as kchp, \
                 tc.tile_pool(name="vch", bufs=1) as vchp, \
                 tc.tile_pool(name="vaug", bufs=1) as vaugp, \
                 tc.tile_pool(name="ep", bufs=2, space="PSUM") as epp, \
                 tc.tile_pool(name="avp", bufs=2, space="PSUM") as avpp, \
                 tc.tile_pool(name="bcp", bufs=1, space="PSUM") as bcpp, \
                 tc.tile_pool(name="esb", bufs=6) as esbp:

                # gathered k: [r][6 ct][128][512]; v: [r][4 lt][128][768]
                k_ch = kchp.tile([128, 4, 6, TLOC], BF16)
                v_ch = vchp.tile([128, 4, 4, C], BF16)
                v_aug = vaugp.tile([128, NKT, 12 * 65], BF16)

                for r in range(4):
                    nc.sync.dma_start(
                        out=k_ch[:, r, :, :],
                        in_=kvk_all[r].rearrange("(ct p t) -> p ct t",
                                                 p=128, t=TLOC))
                for r in range(4):
                    nc.sync.dma_start(
                        out=v_ch[:, r, :, :],
                        in_=kvv_all[r].rearrange("(tt p c) -> p tt c",
                                                 p=128, c=C))
                # assemble v_aug: per rank, two chunk-pairs of v tiles
                va4 = v_aug[:].rearrange("p kt (h e) -> p kt h e", e=65)
                nc.vector.memset(va4[:, :, :, 64:65], 1.0)
                for r in range(4):
                    for half, kt0 in ((0, 2 * r), (1, 14 - 2 * r)):
                        vsrc = v_ch[:, r, 2 * half:2 * half + 2, :].rearrange(
                            "p l (h e) -> p l h e", e=64)
                        nc.vector.tensor_copy(
                            va4[:, kt0:kt0 + 2, :, 0:64], vsrc)

                def k_ap_of(kt, h):
                    ck = kt // 2
                    r = _rank_of_chunk(ck)
                    loc = _loc_of_chunk(ck) + (kt % 2) * 128
                    return k_ch[(h % 2) * 64:(h % 2) * 64 + 64, r, h // 2,
                                loc:loc + 128]

                def finalize_head(h, pav):
                    d_sb = small.tile([1, TLOC], F32, tag="dsb", name="dsb")
                    nc.vector.tensor_copy(d_sb, pav[64:65, :])
                    pb = bcpp.tile([64, TLOC], F32, tag="bc", name="pbc")
                    nc.tensor.matmul(pb, lhsT=ones64, rhs=d_sb,
                                     start=True, stop=True)
                    b_sb = small.tile([64, TLOC], F32, tag="bsb", name="bsb")
                    nc.vector.reciprocal_approx_fast(out=b_sb, in_=pb)
                    nc.vector.tensor_mul(yT_all[:, h, :], pav[0:64, :], b_sb)

                def emit_av(pend):
                    h, e_sb, (g0, gn) = pend
                    pav = pavs[h]
                    off0 = _slot_off(g0)
                    for i in range(gn):
                        kt = g0 + i
                        w = _slot_w(kt)
                        so = _slot_off(kt) - off0
                        out = pav if w == 512 else pav[:, CHUNK:TLOC]
                        nc.tensor.matmul(
                            out, lhsT=v_aug[:, kt, h * 65:(h + 1) * 65],
                            rhs=e_sb[:, so:so + w],
                            start=(kt == 0), stop=(kt == NKT - 1),
                            skip_group_check=True)

                pavs = {}
                pends = []
                for h in range(12):
                    q_full = qT[(h % 2) * 64:(h % 2) * 64 + 64, h // 2, :]
                    q_c1 = qT[(h % 2) * 64:(h % 2) * 64 + 64, h // 2,
                              CHUNK:TLOC]
                    pavs[h] = avpp.tile([65, TLOC], F32, tag="av",
                                        name=f"pav_{h}")
                    for (g0, gn) in GROUPS:
                        pe = epp.tile([128, 1024], F32, tag="e")
                        off0 = _slot_off(g0)
                        for i in range(gn):
                            kt = g0 + i
                            w = _slot_w(kt)
                            so = _slot_off(kt) - off0
                            nc.tensor.matmul(
                                pe[:, so:so + w], lhsT=k_ap_of(kt, h),
                                rhs=(q_full if w == 512 else q_c1),
                                start=True, stop=True)
                        e_sb = esbp.tile([128, 1024], BF16, tag="esb")
                        nc.scalar.activation(
                            out=e_sb, in_=pe,
                            func=mybir.ActivationFunctionType.Exp)
                        nc.gpsimd.tensor_mul(
                            e_sb, e_sb, masks_sb[:, off0:off0 + 1024])
                        pends.append((h, e_sb, (g0, gn)))
                        if len(pends) > 4:
                            pend = pends.pop(0)
                            emit_av(pend)
                            if pend[2][0] + pend[2][1] == NKT:
                                finalize_head(pend[0], pavs[pend[0]])
                                del pavs[pend[0]]
                for pend in pends:
                    emit_av(pend)
                    if pend[2][0] + pend[2][1] == NKT:
                        finalize_head(pend[0], pavs[pend[0]])
                        del pavs[pend[0]]
                pends = []

            # ---------------- proj + residual + LN2 ----------------
            with tc.tile_pool(name="pp", bufs=2, space="PSUM") as ppp, \
                 tc.tile_pool(name="ln2", bufs=3) as ln2p, \
                 tc.tile_pool(name="tp2", bufs=2, space="PSUM") as tpp2:

                xn2s = []
                for t in range(4):
                    pp = ppp.tile([128, C], F32, tag="pp")
                    for h in range(12):
                        y_ap = yT_all[:, h, t * 128:(t + 1) * 128]
                        nc.tensor.matmul(pp[:, 0:512], lhsT=y_ap,
                                         rhs=wp_sb[:, h, 0:512],
                                         start=(h == 0), stop=(h == 11))
                        nc.tensor.matmul(pp[:, 512:768], lhsT=y_ap,
                                         rhs=wp_sb[:, h, 512:768],
                                         start=(h == 0), stop=(h == 11))
                    nc.vector.tensor_add(x_sb[:, t, :], x_sb[:, t, :], pp)
                    if add_proj_bias:
                        nc.vector.tensor_add(x_sb[:, t, :], x_sb[:, t, :],
                                             bout_sb[:, 0, :])
                    xn2 = ln2p.tile([128, C], BF16, tag="xn2", name="xn2")
                    layernorm_to(ln2p, x_sb[:, t, :], xn2, "2")
                    xn2s.append(xn2)
                for t in range(4):
                    for ct in range(6):
                        pt = tpp2.tile([128, 128], BF16, tag="tp2")
                        nc.tensor.transpose(
                            pt, xn2s[t][:, ct * 128:(ct + 1) * 128], ident)
                        nc.vector.tensor_copy(
                            hT[:, ct, t * 128:(t + 1) * 128], pt)

            # ---------------- MLP ----------------
            with tc.tile_pool(name="mlp", bufs=1) as mlpp, \
                 tc.tile_pool(name="wfc", bufs=6) as wfcp, \
                 tc.tile_pool(name="wfc2", bufs=6) as wfc2p, \
                 tc.tile_pool(name="osb", bufs=3) as osbp:

                gT = mlpp.tile([128, 24, TLOC], BF16)
                wfc_t = wfc_ext.ap().rearrange("(k p) n -> p k n", p=128)
                with tc.tile_pool(name="fcp", bufs=2, space="PSUM") as fcpp:
                    for m in range(24):
                        wt = wfcp.tile([128, 6, 128], BF16, tag="wfc")
                        nc.sync.dma_start(
                            out=wt, in_=wfc_t[:, :, m * 128:(m + 1) * 128])
                        pf = fcpp.tile([128, TLOC], F32, tag="fc")
                        for k in range(6):
                            nc.tensor.matmul(pf, lhsT=wt[:, k, :],
                                             rhs=hT[:, k, :],
                                             start=(k == 0), stop=(k == 5))
                        nc.scalar.activation(
                            out=gT[:, m, :], in_=pf,
                            func=mybir.ActivationFunctionType.Gelu_apprx_tanh,
                            bias=bfc_sb[:, m:m + 1])

                wfc2_t = wfc2_ext.ap().rearrange("(k p) n -> k p n", p=128)
                with tc.tile_pool(name="f2p", bufs=1, space="PSUM") as f2pp:
                    pf2s = [f2pp.tile([128, C], F32, tag=f"f2_{t}",
                                      name=f"pf2_{t}")
                            for t in range(4)]
                    for k in range(24):
                        wt2 = wfc2p.tile([128, C], BF16, tag="wfc2")
                        nc.sync.dma_start(out=wt2, in_=wfc2_t[k])
                        for t in range(4):
                            nc.tensor.matmul(
                                pf2s[t][:, 0:512],
                                lhsT=gT[:, k, t * 128:(t + 1) * 128],
                                rhs=wt2[:, 0:512],
                                start=(k == 0), stop=(k == 23))
                            nc.tensor.matmul(
                                pf2s[t][:, 512:768],
                                lhsT=gT[:, k, t * 128:(t + 1) * 128],
                                rhs=wt2[:, 512:768],
                                start=(k == 0), stop=(k == 23))
                    for t in range(4):
                        o_sb = osbp.tile([128, C], F32, tag="osb", name="osb")
                        nc.vector.tensor_add(o_sb, x_sb[:, t, :], pf2s[t])
                        if add_fc2_bias:
                            nc.vector.tensor_add(o_sb, o_sb, bout_sb[:, 1, :])
                        nc.sync.dma_start(
                            out=out_ext[t * 128:(t + 1) * 128, :], in_=o_sb)

    nc.compile()
    return nc


def _preprocess(inputs):
    f = lambda k: np.asarray(inputs[k], np.float32)
    x = f("x"); w_attn = f("w_attn"); b_attn = f("b_attn")
    w_proj = f("w_proj"); b_proj = f("b_proj")
    w_fc = f("w_fc"); b_fc = f("b_fc"); w_fc2 = f("w_fc2"); b_fc2 = f("b_fc2")
    ln1_g = f("ln1_g"); ln1_b = f("ln1_b"); ln2_g = f("ln2_g"); ln2_b = f("ln2_b")

    w_attn_eff = ln1_g[:, None] * w_attn
    b_attn_eff = b_attn + ln1_b @ w_attn
    s = 1.0 / np.sqrt(HD)
    w_q = w_attn_eff[:, 0:C] * s
    w_k = w_attn_eff[:, C:2 * C]
    w_v = w_attn_eff[:, 2 * C:3 * C]
    b_q = b_attn_eff[0:C] * s
    b_k = b_attn_eff[C:2 * C]
    b_v = b_attn_eff[2 * C:3 * C]
    b_proj_eff = b_proj + b_v @ w_proj
    w_fc_eff = ln2_g[:, None] * w_fc
    b_fc_eff = b_fc + ln2_b @ w_fc

    wq16 = np.ascontiguousarray(w_q.astype(BF))
    wk16 = np.ascontiguousarray(w_k.astype(BF))
    wv16 = np.ascontiguousarray(w_v.astype(BF))
    wp16 = np.ascontiguousarray(w_proj.reshape(12, 64, C).astype(BF))
    wfc16 = np.ascontiguousarray(w_fc_eff.astype(BF))
    wfc216 = np.ascontiguousarray(w_fc2.astype(BF))

    bqk = np.stack([b_q, b_k]).astype(np.float32)
    bout = np.stack([b_proj_eff, b_fc2]).astype(np.float32)

    flags = (bool(np.any(bqk != 0)), bool(np.any(b_proj_eff != 0)),
             bool(np.any(b_fc2 != 0)))

    # mask slab [128, 6144] per core group j; kt<8 slots cover both q-chunks
    kpos = np.arange(128)
    qpos = np.arange(CHUNK)
    masks = np.zeros((4, 128, MASK_W), np.float32)
    for j in range(4):
        for kt in range(NKT):
            gk = kt * 128 + kpos[:, None]
            off = _slot_off(kt)
            if kt < 8:
                gq0 = j * CHUNK + qpos[None, :]
                gq1 = (7 - j) * CHUNK + qpos[None, :]
                masks[j, :, off:off + 256] = (gq0 >= gk)
                masks[j, :, off + 256:off + 512] = (gq1 >= gk)
            else:
                gq1 = (7 - j) * CHUNK + qpos[None, :]
                masks[j, :, off:off + 256] = (gq1 >= gk)
    masks16 = masks.astype(BF)

    in_maps = []
    for c in range(NCORES):
        b, j = c // 4, c % 4
        x_loc = np.concatenate(
            [x[b, j * CHUNK:(j + 1) * CHUNK],
             x[b, (7 - j) * CHUNK:(8 - j) * CHUNK]]).astype(np.float32)
        in_maps.append({
            "x": np.ascontiguousarray(x_loc),
            "wq": wq16, "wk": wk16, "wv": wv16, "wp": wp16,
            "wfc": wfc16, "wfc2": wfc216,
            "masks": np.ascontiguousarray(masks16[j]),
            "bqk": bqk, "bfc": b_fc_eff.astype(np.float32), "bout": bout,
        })
    return in_maps, flags


def kernel(**inputs):
    global LAST_EXEC_NS, LAST_RESULTS
    in_maps, flags = _preprocess(inputs)
    if flags not in _CACHE:
        _CACHE[flags] = _build(*flags)
    nc = _CACHE[flags]
    trace = bool(os.environ.get("BASS_KERNEL_TRACE"))
    res = run_bass_kernel_spmd(nc, in_maps, core_ids=list(range(NCORES)),
                               trace=trace)
    LAST_EXEC_NS = res.exec_time_ns
    LAST_RESULTS = res
    out = np.empty((B, T, C), np.float32)
    for c in range(NCORES):
        b, j = c // 4, c % 4
        o = res.results[c]["out"]
        out[b, j * CHUNK:(j + 1) * CHUNK] = o[0:CHUNK]
        out[b, (7 - j) * CHUNK:(8 - j) * CHUNK] = o[CHUNK:TLOC]
    return out


# revision 19
# speedup vs baseline: 1.1590x; 1.0372x over previous
"""GPT-2 style transformer block on 8 TRN2 NeuronCores.

Sharding: token-data-parallel. Each batch's 2048 tokens are split into 8
chunks of 256; core c owns batch c//4 and chunks {j, 7-j} (j = c%4) so
causal attention work is balanced. QKV/proj/MLP/LN are purely local; the
only collectives are two AllGathers (k^T, then v) within each 4-core
batch group. Causality is enforced with per-core 0/1 mask tensors so all
cores run one identical SPMD graph (uniform loop bounds; masks zero the
beyond-causal tiles, which also makes the per-core graphs j-independent).

Matmuls run in bf16 (f32 PSUM accumulation); LN/softmax/residuals in f32.
LN affine params are folded into the following matmul weights host-side;
the attention 1/sqrt(hd) scale is folded into w_q; the v-bias is folded
into the proj bias via the softmax-rows-sum-to-one identity. Softmax is
computed without max-subtraction (scores are O(1) here, exp cannot
overflow in f32) as exp(s) normalized by a denominator obtained for free
as an extra ones-column in the av matmul. Both local q-chunks share one
[65, 512] av accumulator per head; key tiles 0..7 are scored against all
512 local queries in one matmul, tiles 8..15 only against q-chunk 1.
"""

import os
import sys

sys.path.insert(0, "/opt/trn_rl_repo")

import numpy as np
import ml_dtypes

import concourse.bass as bass
import concourse.tile as tile
from concourse import bacc, mybir
from concourse.bass_utils import run_bass_kernel_spmd
from concourse.masks import make_identity

F32 = mybir.dt.float32
BF16 = mybir.dt.bfloat16
BF = ml_dtypes.bfloat16

B, T, C, H, HD = 2, 2048, 768, 12, 64
EPS = 1e-5
NCORES = 8
CHUNK = 256            # global chunk size (tokens)
TLOC = 512             # local tokens per core (2 chunks)
NKT = T // 128         # 16 key tiles per batch
CC = T // CHUNK        # 8 chunks per batch

# e-slot layout: kt<8 -> 512 wide (both q-chunks), kt>=8 -> 256 (q-chunk 1)
def _slot_off(kt):
    return kt * 512 if kt < 8 else 4096 + (kt - 8) * 256


def _slot_w(kt):
    return 512 if kt < 8 else 256


MASK_W = 8 * 512 + 8 * 256   # 6144
# exp groups: contiguous 1024-col spans of the slot layout
GROUPS = [(0, 2), (2, 2), (4, 2), (6, 2), (8, 4), (12, 4)]

KT_ELEMS = 6 * 128 * TLOC          # k^T bounce: [6 ct][128 p][512 t]
V_ELEMS = 4 * 128 * C              # v bounce:   [4 tt][128 p][768 c]

LAST_EXEC_NS = None
LAST_RESULTS = None
_CACHE = {}


def _rank_of_chunk(ck):
    return ck if ck < 4 else 7 - ck


def _loc_of_chunk(ck):
    return 0 if ck < 4 else CHUNK


def _build(add_qk_bias, add_proj_bias, add_fc2_bias):
    nc = bacc.Bacc("TRN2", target_bir_lowering=False, debug=False,
                   num_devices=NCORES)

    x_ext = nc.dram_tensor("x", [TLOC, C], F32, kind="ExternalInput")
    wq_ext = nc.dram_tensor("wq", [C, C], BF16, kind="ExternalInput")
    wk_ext = nc.dram_tensor("wk", [C, C], BF16, kind="ExternalInput")
    wv_ext = nc.dram_tensor("wv", [C, C], BF16, kind="ExternalInput")
    wp_ext = nc.dram_tensor("wp", [12, 128, C], BF16, kind="ExternalInput")
    wfc_ext = nc.dram_tensor("wfc", [C, 4 * C], BF16, kind="ExternalInput")
    wfc2_ext = nc.dram_tensor("wfc2", [4 * C, C], BF16, kind="ExternalInput")
    masks_ext = nc.dram_tensor("masks", [128, MASK_W], BF16,
                               kind="ExternalInput")
    bqk_ext = nc.dram_tensor("bqk", [2, C], F32, kind="ExternalInput")
    bfc_ext = nc.dram_tensor("bfc", [4 * C], F32, kind="ExternalInput")
    bout_ext = nc.dram_tensor("bout", [2, C], F32, kind="ExternalInput")
    out_ext = nc.dram_tensor("out", [TLOC, C], F32, kind="ExternalOutput")

    with tile.TileContext(nc) as tc:
        with tc.tile_pool(name="dram", bufs=1, space="DRAM") as dram, \
             tc.tile_pool(name="singles", bufs=1) as singles, \
             tc.tile_pool(name="persist", bufs=1) as persist, \
             tc.tile_pool(name="small", bufs=3) as small:

            kvk_in = dram.tile([KT_ELEMS], BF16)
            kvk_all = dram.tile([4, KT_ELEMS], BF16)
            kvv_in = dram.tile([V_ELEMS], BF16)
            kvv_all = dram.tile([4, V_ELEMS], BF16)

            ident = singles.tile([128, 128], BF16)
            make_identity(nc, ident)
            eps_sb = singles.tile([128, 1], F32)
            nc.vector.memset(eps_sb, EPS)
            ones_pad = singles.tile([128, 64], F32)
            nc.vector.memset(ones_pad, 0.0)
            nc.vector.memset(ones_pad[0:1, :], 1.0)
            d_sb = singles.tile([128, TLOC], F32)
            nc.vector.memset(d_sb, 1.0)

            wq_sb = persist.tile([128, 6, C], BF16)
            wk_sb = persist.tile([128, 6, C], BF16)
            wv_sb = persist.tile([128, 6, C], BF16)
            wp_sb = persist.tile([128, 12, C], BF16)
            for sb, ext in ((wq_sb, wq_ext), (wk_sb, wk_ext),
                            (wv_sb, wv_ext)):
                nc.sync.dma_start(
                    out=sb, in_=ext.ap().rearrange("(ct p) c -> p ct c", p=128))
            nc.sync.dma_start(
                out=wp_sb, in_=wp_ext.ap().rearrange("h p c -> p h c"))

            bqk_sb = singles.tile([128, 2, 6], F32)
            if add_qk_bias:
                nc.sync.dma_start(
                    out=bqk_sb,
                    in_=bqk_ext.ap().rearrange("b (m p) -> p b m", p=128))
            bfc_sb = singles.tile([128, 24], F32)
            nc.sync.dma_start(
                out=bfc_sb, in_=bfc_ext.ap().rearrange("(m p) -> p m", p=128))
            bout_sb = singles.tile([128, 2, C], F32)
            if add_proj_bias or add_fc2_bias:
                bc = bout_ext.ap()
                nc.sync.dma_start(
                    out=bout_sb,
                    in_=bass.AP(tensor=bc.tensor, offset=bc.offset,
                                ap=[[0, 128], bc.ap[0], bc.ap[1]]))

            masks_sb = persist.tile([128, MASK_W], BF16)
            nc.sync.dma_start(out=masks_sb, in_=masks_ext.ap())

            x_sb = persist.tile([128, 4, C], F32)     # local x, becomes xmid
            hT = persist.tile([128, 6, TLOC], BF16)   # h^T, reused for h2^T
            qT = persist.tile([128, 12, TLOC], BF16)
            nc.vector.memset(qT[:], 0.0)
            yT_all = persist.tile([128, 12, TLOC], BF16)
            nc.vector.memset(yT_all[64:128, :, :], 0.0)

            def layernorm_to(pool, xt, dst, tagsuf):
                stats = pool.tile([128, 3, 6], F32, tag="st" + tagsuf,
                                  name="st" + tagsuf)
                for sg in range(3):
                    nc.vector.bn_stats(out=stats[:, sg, :],
                                       in_=xt[:, sg * 256:(sg + 1) * 256])
                mv = pool.tile([128, 2], F32, tag="mv" + tagsuf,
                               name="mv" + tagsuf)
                nc.vector.bn_aggr(out=mv, in_=stats)
                nc.scalar.activation(out=mv[:, 1:2], in_=mv[:, 1:2],
                                     func=mybir.ActivationFunctionType.Sqrt,
                                     bias=eps_sb)
                nc.vector.reciprocal(out=mv[:, 1:2], in_=mv[:, 1:2])
                nc.vector.tensor_scalar(out=dst, in0=xt,
                                        scalar1=mv[:, 0:1], scalar2=mv[:, 1:2],
                                        op0=mybir.AluOpType.subtract,
                                        op1=mybir.AluOpType.mult)

            # ---------------- LN1 + transpose + QKV + AGs ----------------
            with tc.tile_pool(name="ln", bufs=3) as lnp, \
                 tc.tile_pool(name="tp", bufs=2, space="PSUM") as tpp, \
                 tc.tile_pool(name="qkp", bufs=2, space="PSUM") as qkp, \
                 tc.tile_pool(name="vp", bufs=2, space="PSUM") as vpp, \
                 tc.tile_pool(name="vsb", bufs=1) as vsbp:

                kT = vsbp.tile([128, 6, TLOC], BF16)
                for t in range(4):
                    nc.sync.dma_start(out=x_sb[:, t, :],
                                      in_=x_ext[t * 128:(t + 1) * 128, :])
                    xn = lnp.tile([128, C], BF16, tag="xn")
                    layernorm_to(lnp, x_sb[:, t, :], xn, "1")
                    for ct in range(6):
                        pt = tpp.tile([128, 128], BF16, tag="tp")
                        nc.tensor.transpose(
                            pt, xn[:, ct * 128:(ct + 1) * 128], ident)
                        nc.vector.tensor_copy(
                            hT[:, ct, t * 128:(t + 1) * 128], pt)

                # k^T first: it feeds the first collective
                for m in range(6):
                    ps = qkp.tile([128, TLOC], F32, tag="qk")
                    for k in range(6):
                        nc.tensor.matmul(
                            ps, lhsT=wk_sb[:, k, m * 128:(m + 1) * 128],
                            rhs=hT[:, k, :], start=(k == 0), stop=(k == 5))
                    if add_qk_bias:
                        nc.vector.tensor_scalar_add(
                            out=kT[:, m, :], in0=ps,
                            scalar1=bqk_sb[:, 1, m:m + 1])
                    else:
                        nc.vector.tensor_copy(kT[:, m, :], ps)
                nc.sync.dma_start(
                    out=kvk_in[:].rearrange("(ct p t) -> p ct t", p=128, t=TLOC),
                    in_=kT)
                nc.gpsimd.collective_compute(
                    "AllGather", mybir.AluOpType.bypass,
                    replica_groups=[[0, 1, 2, 3], [4, 5, 6, 7]],
                    ins=[kvk_in[:].opt()], outs=[kvk_all[:].opt()])

                v_sb = vsbp.tile([128, 4, C], BF16)
                for tt in range(4):
                    pv = vpp.tile([128, C], F32, tag="v")
                    for k in range(6):
                        nc.tensor.matmul(
                            pv[:, 0:512],
                            lhsT=hT[:, k, tt * 128:(tt + 1) * 128],
                            rhs=wv_sb[:, k, 0:512],
                            start=(k == 0), stop=(k == 5))
                        nc.tensor.matmul(
                            pv[:, 512:768],
                            lhsT=hT[:, k, tt * 128:(tt + 1) * 128],
                            rhs=wv_sb[:, k, 512:768],
                            start=(k == 0), stop=(k == 5))
                    nc.vector.tensor_copy(v_sb[:, tt, :], pv)
                nc.sync.dma_start(
                    out=kvv_in[:].rearrange("(tt p c) -> p tt c", p=128, c=C),
                    in_=v_sb)
                nc.gpsimd.collective_compute(
                    "AllGather", mybir.AluOpType.bypass,
                    replica_groups=[[0, 1, 2, 3], [4, 5, 6, 7]],
                    ins=[kvv_in[:].opt()], outs=[kvv_all[:].opt()])

                for m in range(6):
                    ps = qkp.tile([128, TLOC], F32, tag="qk")
                    for k in range(6):
                        nc.tensor.matmul(
                            ps, lhsT=wq_sb[:, k, m * 128:(m + 1) * 128],
                            rhs=hT[:, k, :], start=(k == 0), stop=(k == 5))
                    for par in range(2):
                        h = 2 * m + par
                        sl = slice(par * 64, par * 64 + 64)
                        if add_qk_bias:
                            nc.vector.tensor_scalar_add(
                                out=qT[sl, h, :], in0=ps[sl, :],
                                scalar1=bqk_sb[sl, 0, m:m + 1])
                        else:
                            nc.vector.tensor_copy(qT[sl, h, :], ps[sl, :])

            # ---------------- attention ----------------
            with tc.tile_pool(name="kch", bufs=1) as kchp, \
                 tc.tile_pool(name="vch", bufs=1) as vchp, \
                 tc.tile_pool(name="vaug", bufs=1) as vaugp, \
                 tc.tile_pool(name="ep", bufs=2, space="PSUM") as epp, \
                 tc.tile_pool(name="avp", bufs=2, space="PSUM") as avpp, \
                 tc.tile_pool(name="bcp", bufs=1, space="PSUM") as bcpp, \
                 tc.tile_pool(name="esb", bufs=6) as esbp:

                # gathered k: [r][6 ct][128][512]; v: [r][4 lt][128][768]
                k_ch = kchp.tile([128, 4, 6, TLOC], BF16)
                v_ch = vchp.tile([128, 4, 4, C], BF16)
                v_aug = vaugp.tile([128, NKT, 12 * 65], BF16)

                for r in range(4):
                    nc.sync.dma_start(
                        out=k_ch[:, r, :, :],
                        in_=kvk_all[r].rearrange("(ct p t) -> p ct t",
                                                 p=128, t=TLOC))
                for r in range(4):
                    nc.sync.dma_start(
                        out=v_ch[:, r, :, :],
                        in_=kvv_all[r].rearrange("(tt p c) -> p tt c",
                                                 p=128, c=C))
                # assemble v_aug: per rank, two chunk-pairs of v tiles
                va4 = v_aug[:].rearrange("p kt (h e) -> p kt h e", e=65)
                nc.vector.memset(va4[:, :, :, 64:65], 1.0)
                for r in range(4):
                    for half, kt0 in ((0, 2 * r), (1, 14 - 2 * r)):
                        vsrc = v_ch[:, r, 2 * half:2 * half + 2, :].rearrange(
                            "p l (h e) -> p l h e", e=64)
                        nc.vector.tensor_copy(
                            va4[:, kt0:kt0 + 2, :, 0:64], vsrc)

                def k_ap_of(kt, h):
                    ck = kt // 2
                    r = _rank_of_chunk(ck)
                    loc = _loc_of_chunk(ck) + (kt % 2) * 128
                    return k_ch[:, r, h // 2, loc:loc + 128]

                def finalize_head(h, pav):
                    nc.vector.tensor_copy(d_sb[0:1, :], pav[64:65, :])
                    pb = bcpp.tile([64, TLOC], F32, tag="bc", name="pbc")
                    nc.tensor.matmul(pb, lhsT=ones_pad, rhs=d_sb,
                                     start=True, stop=True)
                    b_sb = small.tile([64, TLOC], F32, tag="bsb", name="bsb")
                    nc.vector.reciprocal_approx_fast(out=b_sb, in_=pb)
                    nc.vector.tensor_mul(yT_all[0:64, h, :], pav[0:64, :], b_sb)

                def emit_av(pend):
                    h, e_sb, (g0, gn) = pend
                    pav = pavs[h]
                    off0 = _slot_off(g0)
                    for i in range(gn):
                        kt = g0 + i
                        w = _slot_w(kt)
                        so = _slot_off(kt) - off0
                        out = pav if w == 512 else pav[:, CHUNK:TLOC]
                        nc.tensor.matmul(
                            out, lhsT=v_aug[:, kt, h * 65:(h + 1) * 65],
                            rhs=e_sb[:, so:so + w],
                            start=(kt == 0), stop=(kt == NKT - 1),
                            skip_group_check=True)

                pavs = {}
                pends = []
                for h in range(12):
                    q_full = qT[:, h, :]
                    q_c1 = qT[:, h, CHUNK:TLOC]
                    pavs[h] = avpp.tile([65, TLOC], F32, tag="av",
                                        name=f"pav_{h}")
                    for (g0, gn) in GROUPS:
                        pe = epp.tile([128, 1024], F32, tag="e")
                        off0 = _slot_off(g0)
                        for i in range(gn):
                            kt = g0 + i
                            w = _slot_w(kt)
                            so = _slot_off(kt) - off0
                            nc.tensor.matmul(
                                pe[:, so:so + w], lhsT=k_ap_of(kt, h),
                                rhs=(q_full if w == 512 else q_c1),
                                start=True, stop=True)
                        e_sb = esbp.tile([128, 1024], BF16, tag="esb")
                        nc.scalar.activation(
                            out=e_sb, in_=pe,
                            func=mybir.ActivationFunctionType.Exp)
                        nc.gpsimd.tensor_mul(
                            e_sb, e_sb, masks_sb[:, off0:off0 + 1024])
                        pends.append((h, e_sb, (g0, gn)))
                        if len(pends) > 4:
                            pend = pends.pop(0)
                            emit_av(pend)
                            if pend[2][0] + pend[2][1] == NKT:
                                finalize_head(pend[0], pavs[pend[0]])
                                del pavs[pend[0]]
                for pend in pends:
                    emit_av(pend)
                    if pend[2][0] + pend[2][1] == NKT:
                        finalize_head(pend[0], pavs[pend[0]])
                        del pavs[pend[0]]
                pends = []

            # ---------------- proj + residual + LN2 ----------------
            with tc.tile_pool(name="pp", bufs=2, space="PSUM") as ppp, \
                 tc.tile_pool(name="ln2", bufs=3) as ln2p, \
                 tc.tile_pool(name="tp2", bufs=2, space="PSUM") as tpp2:

                xn2s = []
                for t in range(4):
                    pp = ppp.tile([128, C], F32, tag="pp")
                    for h in range(12):
                        y_ap = yT_all[:, h, t * 128:(t + 1) * 128]
                        nc.tensor.matmul(pp[:, 0:512], lhsT=y_ap,
                                         rhs=wp_sb[:, h, 0:512],
                                         start=(h == 0), stop=(h == 11))
                        nc.tensor.matmul(pp[:, 512:768], lhsT=y_ap,
                                         rhs=wp_sb[:, h, 512:768],
                                         start=(h == 0), stop=(h == 11))
                    nc.vector.tensor_add(x_sb[:, t, :], x_sb[:, t, :], pp)
                    if add_proj_bias:
                        nc.vector.tensor_add(x_sb[:, t, :], x_sb[:, t, :],
                                             bout_sb[:, 0, :])
                    xn2 = ln2p.tile([128, C], BF16, tag="xn2", name="xn2")
                    layernorm_to(ln2p, x_sb[:, t, :], xn2, "2")
                    xn2s.append(xn2)
                for t in range(4):
                    for ct in range(6):
                        pt = tpp2.tile([128, 128], BF16, tag="tp2")
                        nc.tensor.transpose(
                            pt, xn2s[t][:, ct * 128:(ct + 1) * 128], ident)
                        nc.vector.tensor_copy(
                            hT[:, ct, t * 128:(t + 1) * 128], pt)

            # ---------------- MLP ----------------
            with tc.tile_pool(name="mlp", bufs=1) as mlpp, \
                 tc.tile_pool(name="wfc", bufs=6) as wfcp, \
                 tc.tile_pool(name="wfc2", bufs=6) as wfc2p, \
                 tc.tile_pool(name="osb", bufs=3) as osbp:

                gT = mlpp.tile([128, 24, TLOC], BF16)
                wfc_t = wfc_ext.ap().rearrange("(k p) n -> p k n", p=128)
                with tc.tile_pool(name="fcp", bufs=2, space="PSUM") as fcpp:
                    for m in range(24):
                        wt = wfcp.tile([128, 6, 128], BF16, tag="wfc")
                        nc.sync.dma_start(
                            out=wt, in_=wfc_t[:, :, m * 128:(m + 1) * 128])
                        pf = fcpp.tile([128, TLOC], F32, tag="fc")
                        for k in range(6):
                            nc.tensor.matmul(pf, lhsT=wt[:, k, :],
                                             rhs=hT[:, k, :],
                                             start=(k == 0), stop=(k == 5))
                        nc.scalar.activation(
                            out=gT[:, m, :], in_=pf,
                            func=mybir.ActivationFunctionType.Gelu_apprx_tanh,
                            bias=bfc_sb[:, m:m + 1])

                wfc2_t = wfc2_ext.ap().rearrange("(k p) n -> k p n", p=128)
                with tc.tile_pool(name="f2p", bufs=1, space="PSUM") as f2pp:
                    pf2s = [f2pp.tile([128, C], F32, tag=f"f2_{t}",
                                      name=f"pf2_{t}")
                            for t in range(4)]
                    for k in range(24):
                        wt2 = wfc2p.tile([128, C], BF16, tag="wfc2")
                        nc.sync.dma_start(out=wt2, in_=wfc2_t[k])
                        for t in range(4):
                            nc.tensor.matmul(
                                pf2s[t][:, 0:512],
                                lhsT=gT[:, k, t * 128:(t + 1) * 128],
                                rhs=wt2[:, 0:512],
                                start=(k == 0), stop=(k == 23))
                            nc.tensor.matmul(
                                pf2s[t][:, 512:768],
                                lhsT=gT[:, k, t * 128:(t + 1) * 128],
                                rhs=wt2[:, 512:768],
                                start=(k == 0), stop=(k == 23))
                    for t in range(4):
                        o_sb = osbp.tile([128, C], F32, tag="osb", name="osb")
                        nc.vector.tensor_add(o_sb, x_sb[:, t, :], pf2s[t])
                        if add_fc2_bias:
                            nc.vector.tensor_add(o_sb, o_sb, bout_sb[:, 1, :])
                        nc.sync.dma_start(
                            out=out_ext[t * 128:(t + 1) * 128, :], in_=o_sb)

    nc.compile()
    return nc


def _preprocess(inputs):
    f = lambda k: np.asarray(inputs[k], np.float32)
    x = f("x"); w_attn = f("w_attn"); b_attn = f("b_attn")
    w_proj = f("w_proj"); b_proj = f("b_proj")
    w_fc = f("w_fc"); b_fc = f("b_fc"); w_fc2 = f("w_fc2"); b_fc2 = f("b_fc2")
    ln1_g = f("ln1_g"); ln1_b = f("ln1_b"); ln2_g = f("ln2_g"); ln2_b = f("ln2_b")

    w_attn_eff = ln1_g[:, None] * w_attn
    b_attn_eff = b_attn + ln1_b @ w_attn
    s = 1.0 / np.sqrt(HD)
    w_q = w_attn_eff[:, 0:C] * s
    w_k = w_attn_eff[:, C:2 * C]
    w_v = w_attn_eff[:, 2 * C:3 * C]
    b_q = b_attn_eff[0:C] * s
    b_k = b_attn_eff[C:2 * C]
    b_v = b_attn_eff[2 * C:3 * C]
    b_proj_eff = b_proj + b_v @ w_proj
    w_fc_eff = ln2_g[:, None] * w_fc
    b_fc_eff = b_fc + ln2_b @ w_fc

    wq16 = np.ascontiguousarray(w_q.astype(BF))
    wk16 = np.ascontiguousarray(w_k.astype(BF))
    wv16 = np.ascontiguousarray(w_v.astype(BF))
    wp_pad = np.zeros((12, 128, C), np.float32)
    wp_pad[:, 0:64, :] = w_proj.reshape(12, 64, C)
    wp16 = np.ascontiguousarray(wp_pad.astype(BF))
    wfc16 = np.ascontiguousarray(w_fc_eff.astype(BF))
    wfc216 = np.ascontiguousarray(w_fc2.astype(BF))

    bqk = np.stack([b_q, b_k]).astype(np.float32)
    bout = np.stack([b_proj_eff, b_fc2]).astype(np.float32)

    flags = (bool(np.any(bqk != 0)), bool(np.any(b_proj_eff != 0)),
             bool(np.any(b_fc2 != 0)))

    # mask slab [128, 6144] per core group j; kt<8 slots cover both q-chunks
    kpos = np.arange(128)
    qpos = np.arange(CHUNK)
    masks = np.zeros((4, 128, MASK_W), np.float32)
    for j in range(4):
        for kt in range(NKT):
            gk = kt * 128 + kpos[:, None]
            off = _slot_off(kt)
            if kt < 8:
                gq0 = j * CHUNK + qpos[None, :]
                gq1 = (7 - j) * CHUNK + qpos[None, :]
                masks[j, :, off:off + 256] = (gq0 >= gk)
                masks[j, :, off + 256:off + 512] = (gq1 >= gk)
            else:
                gq1 = (7 - j) * CHUNK + qpos[None, :]
                masks[j, :, off:off + 256] = (gq1 >= gk)
    masks16 = masks.astype(BF)

    in_maps = []
    for c in range(NCORES):
        b, j = c // 4, c % 4
        x_loc = np.concatenate(
            [x[b, j * CHUNK:(j + 1) * CHUNK],
             x[b, (7 - j) * CHUNK:(8 - j) * CHUNK]]).astype(np.float32)
        in_maps.append({
            "x": np.ascontiguousarray(x_loc),
            "wq": wq16, "wk": wk16, "wv": wv16, "wp": wp16,
            "wfc": wfc16, "wfc2": wfc216,
            "masks": np.ascontiguousarray(masks16[j]),
            "bqk": bqk, "bfc": b_fc_eff.astype(np.float32), "bout": bout,
        })
    return in_maps, flags


def kernel(**inputs):
    global LAST_EXEC_NS, LAST_RESULTS
    in_maps, flags = _preprocess(inputs)
    if flags not in _CACHE:
        _CACHE[flags] = _build(*flags)
    nc = _CACHE[flags]
    trace = bool(os.environ.get("BASS_KERNEL_TRACE"))
    res = run_bass_kernel_spmd(nc, in_maps, core_ids=list(range(NCORES)),
                               trace=trace)
    LAST_EXEC_NS = res.exec_time_ns
    LAST_RESULTS = res
    out = np.empty((B, T, C), np.float32)
    for c in range(NCORES):
        b, j = c // 4, c % 4
        o = res.results[c]["out"]
        out[b, j * CHUNK:(j + 1) * CHUNK] = o[0:CHUNK]
        out[b, (7 - j) * CHUNK:(8 - j) * CHUNK] = o[CHUNK:TLOC]
    return out


# revision 20
# speedup vs baseline: 1.2984x; 1.1203x over previous
"""GPT-2 style transformer block on 8 TRN2 NeuronCores.

Sharding: token-data-parallel. Each batch's 2048 tokens are split into 8
chunks of 256; core c owns batch c//4 and chunks {j, 7-j} (j = c%4) so
causal attention work is balanced. QKV/proj/MLP/LN are purely local; the
only collectives are two AllGathers (k^T, then v) within each 4-core
batch group. Causality is enforced with per-core 0/1 mask tensors so all
cores run one identical SPMD graph (uniform loop bounds; masks zero the
beyond-causal tiles, which also makes the per-core graphs j-independent).

Matmuls run in bf16 (f32 PSUM accumulation); LN/softmax/residuals in f32.
LN affine params are folded into the following matmul weights host-side;
the attention 1/sqrt(hd) scale is folded into w_q; the v-bias is folded
into the proj bias via the softmax-rows-sum-to-one identity. Softmax is
computed without max-subtraction (scores are O(1) here, exp cannot
overflow in f32) as exp(s) normalized by a denominator obtained for free
as an extra ones-column in the av matmul. Both local q-chunks share one
[65, 512] av accumulator per head; key tiles 0..7 are scored against all
512 local queries in one matmul, tiles 8..15 only against q-chunk 1.
"""

import os
import sys

sys.path.insert(0, "/opt/trn_rl_repo")

import numpy as np
import ml_dtypes

import concourse.bass as bass
import concourse.tile as tile
from concourse import bacc, mybir
from concourse.bass_utils import run_bass_kernel_spmd
from concourse.masks import make_identity

F32 = mybir.dt.float32
BF16 = mybir.dt.bfloat16
BF = ml_dtypes.bfloat16

B, T, C, H, HD = 2, 2048, 768, 12, 64
EPS = 1e-5
NCORES = 8
CHUNK = 256            # global chunk size (tokens)
TLOC = 512             # local tokens per core (2 chunks)
NKT = T // 128         # 16 key tiles per batch
CC = T // CHUNK        # 8 chunks per batch

# e-slot layout: kt<8 -> 512 wide (both q-chunks), kt>=8 -> 256 (q-chunk 1)
def _slot_off(kt):
    return kt * 512 if kt < 8 else 4096 + (kt - 8) * 256


def _slot_w(kt):
    return 512 if kt < 8 else 256


MASK_W = 8 * 512 + 8 * 256   # 6144
# exp groups: contiguous 1024-col spans of the slot layout
GROUPS = [(0, 2), (2, 2), (4, 2), (6, 2), (8, 4), (12, 4)]

KT_ELEMS = 6 * 128 * TLOC          # k^T bounce: [6 ct][128 p][512 t]
V_ELEMS = 4 * 128 * C              # v bounce:   [4 tt][128 p][768 c]

LAST_EXEC_NS = None
LAST_RESULTS = None
_CACHE = {}


def _rank_of_chunk(ck):
    return ck if ck < 4 else 7 - ck


def _loc_of_chunk(ck):
    return 0 if ck < 4 else CHUNK


def _build(add_qk_bias, add_proj_bias, add_fc2_bias):
    nc = bacc.Bacc("TRN2", target_bir_lowering=False, debug=False,
                   num_devices=NCORES)

    x_ext = nc.dram_tensor("x", [TLOC, C], F32, kind="ExternalInput")
    wq_ext = nc.dram_tensor("wq", [C, C], BF16, kind="ExternalInput")
    wk_ext = nc.dram_tensor("wk", [C, C], BF16, kind="ExternalInput")
    wv_ext = nc.dram_tensor("wv", [C, C], BF16, kind="ExternalInput")
    wp_ext = nc.dram_tensor("wp", [12, 128, C], BF16, kind="ExternalInput")
    wfc_ext = nc.dram_tensor("wfc", [C, 4 * C], BF16, kind="ExternalInput")
    wfc2_ext = nc.dram_tensor("wfc2", [4 * C, C], BF16, kind="ExternalInput")
    masks_ext = nc.dram_tensor("masks", [128, MASK_W], BF16,
                               kind="ExternalInput")
    bqk_ext = nc.dram_tensor("bqk", [2, C], F32, kind="ExternalInput")
    bfc_ext = nc.dram_tensor("bfc", [4 * C], F32, kind="ExternalInput")
    bout_ext = nc.dram_tensor("bout", [2, C], F32, kind="ExternalInput")
    out_ext = nc.dram_tensor("out", [TLOC, C], F32, kind="ExternalOutput")

    with tile.TileContext(nc) as tc:
        with tc.tile_pool(name="dram", bufs=1, space="DRAM") as dram, \
             tc.tile_pool(name="singles", bufs=1) as singles, \
             tc.tile_pool(name="persist", bufs=1) as persist, \
             tc.tile_pool(name="small", bufs=3) as small:

            kvk_in = dram.tile([KT_ELEMS], BF16)
            kvk_all = dram.tile([4, KT_ELEMS], BF16)
            kvv_in = dram.tile([V_ELEMS], BF16)
            kvv_all = dram.tile([4, V_ELEMS], BF16)

            ident = singles.tile([128, 128], BF16)
            make_identity(nc, ident)
            eps_sb = singles.tile([128, 1], F32)
            nc.vector.memset(eps_sb, EPS)
            ones_pad = singles.tile([128, 64], F32)
            nc.vector.memset(ones_pad, 0.0)
            nc.vector.memset(ones_pad[0:1, :], 1.0)
            d_sb = singles.tile([128, TLOC], F32)
            nc.vector.memset(d_sb, 1.0)

            wq_sb = persist.tile([128, 6, C], BF16)
            wk_sb = persist.tile([128, 6, C], BF16)
            wv_sb = persist.tile([128, 6, C], BF16)
            wp_sb = persist.tile([128, 12, C], BF16)
            for sb, ext in ((wq_sb, wq_ext), (wk_sb, wk_ext),
                            (wv_sb, wv_ext)):
                nc.sync.dma_start(
                    out=sb, in_=ext.ap().rearrange("(ct p) c -> p ct c", p=128))
            nc.sync.dma_start(
                out=wp_sb, in_=wp_ext.ap().rearrange("h p c -> p h c"))

            bqk_sb = singles.tile([128, 2, 6], F32)
            if add_qk_bias:
                nc.sync.dma_start(
                    out=bqk_sb,
                    in_=bqk_ext.ap().rearrange("b (m p) -> p b m", p=128))
            bfc_sb = singles.tile([128, 24], F32)
            nc.sync.dma_start(
                out=bfc_sb, in_=bfc_ext.ap().rearrange("(m p) -> p m", p=128))
            bout_sb = singles.tile([128, 2, C], F32)
            if add_proj_bias or add_fc2_bias:
                bc = bout_ext.ap()
                nc.sync.dma_start(
                    out=bout_sb,
                    in_=bass.AP(tensor=bc.tensor, offset=bc.offset,
                                ap=[[0, 128], bc.ap[0], bc.ap[1]]))

            masks_sb = persist.tile([128, MASK_W], BF16)
            nc.sync.dma_start(out=masks_sb, in_=masks_ext.ap())

            x_sb = persist.tile([128, 4, C], F32)     # local x, becomes xmid
            hT = persist.tile([128, 6, TLOC], BF16)   # h^T, reused for h2^T
            qT = persist.tile([128, 12, TLOC], BF16)
            nc.vector.memset(qT[:], 0.0)
            yT_all = persist.tile([128, 12, TLOC], BF16)
            nc.vector.memset(yT_all[64:128, :, :], 0.0)

            def layernorm_to(pool, xt, dst, tagsuf):
                stats = pool.tile([128, 3, 6], F32, tag="st" + tagsuf,
                                  name="st" + tagsuf)
                for sg in range(3):
                    nc.vector.bn_stats(out=stats[:, sg, :],
                                       in_=xt[:, sg * 256:(sg + 1) * 256])
                mv = pool.tile([128, 2], F32, tag="mv" + tagsuf,
                               name="mv" + tagsuf)
                nc.vector.bn_aggr(out=mv, in_=stats)
                nc.scalar.activation(out=mv[:, 1:2], in_=mv[:, 1:2],
                                     func=mybir.ActivationFunctionType.Sqrt,
                                     bias=eps_sb)
                nc.vector.reciprocal(out=mv[:, 1:2], in_=mv[:, 1:2])
                nc.vector.tensor_scalar(out=dst, in0=xt,
                                        scalar1=mv[:, 0:1], scalar2=mv[:, 1:2],
                                        op0=mybir.AluOpType.subtract,
                                        op1=mybir.AluOpType.mult)

            # ---------------- LN1 + transpose + QKV + AGs ----------------
            with tc.tile_pool(name="ln", bufs=3) as lnp, \
                 tc.tile_pool(name="tp", bufs=2, space="PSUM") as tpp, \
                 tc.tile_pool(name="qkp", bufs=2, space="PSUM") as qkp, \
                 tc.tile_pool(name="vp", bufs=2, space="PSUM") as vpp, \
                 tc.tile_pool(name="vsb", bufs=1) as vsbp:

                kT = vsbp.tile([128, 6, TLOC], BF16)
                for t in range(4):
                    nc.sync.dma_start(out=x_sb[:, t, :],
                                      in_=x_ext[t * 128:(t + 1) * 128, :])
                    xn = lnp.tile([128, C], BF16, tag="xn")
                    layernorm_to(lnp, x_sb[:, t, :], xn, "1")
                    for ct in range(6):
                        pt = tpp.tile([128, 128], BF16, tag="tp")
                        nc.tensor.transpose(
                            pt, xn[:, ct * 128:(ct + 1) * 128], ident)
                        nc.vector.tensor_copy(
                            hT[:, ct, t * 128:(t + 1) * 128], pt)

                # k^T first: it feeds the first collective
                for m in range(6):
                    ps = qkp.tile([128, TLOC], F32, tag="qk")
                    for k in range(6):
                        nc.tensor.matmul(
                            ps, lhsT=wk_sb[:, k, m * 128:(m + 1) * 128],
                            rhs=hT[:, k, :], start=(k == 0), stop=(k == 5))
                    if add_qk_bias:
                        nc.vector.tensor_scalar_add(
                            out=kT[:, m, :], in0=ps,
                            scalar1=bqk_sb[:, 1, m:m + 1])
                    else:
                        nc.vector.tensor_copy(kT[:, m, :], ps)
                nc.sync.dma_start(
                    out=kvk_in[:].rearrange("(ct p t) -> p ct t", p=128, t=TLOC),
                    in_=kT)
                nc.gpsimd.collective_compute(
                    "AllGather", mybir.AluOpType.bypass,
                    replica_groups=[[0, 1, 2, 3], [4, 5, 6, 7]],
                    ins=[kvk_in[:].opt()], outs=[kvk_all[:].opt()])

                v_sb = vsbp.tile([128, 4, C], BF16)
                for tt in range(4):
                    pv = vpp.tile([128, C], F32, tag="v")
                    for k in range(6):
                        nc.tensor.matmul(
                            pv[:, 0:512],
                            lhsT=hT[:, k, tt * 128:(tt + 1) * 128],
                            rhs=wv_sb[:, k, 0:512],
                            start=(k == 0), stop=(k == 5))
                        nc.tensor.matmul(
                            pv[:, 512:768],
                            lhsT=hT[:, k, tt * 128:(tt + 1) * 128],
                            rhs=wv_sb[:, k, 512:768],
                            start=(k == 0), stop=(k == 5))
                    nc.vector.tensor_copy(v_sb[:, tt, :], pv)
                nc.sync.dma_start(
                    out=kvv_in[:].rearrange("(tt p c) -> p tt c", p=128, c=C),
                    in_=v_sb)
                nc.gpsimd.collective_compute(
                    "AllGather", mybir.AluOpType.bypass,
                    replica_groups=[[0, 1, 2, 3], [4, 5, 6, 7]],
                    ins=[kvv_in[:].opt()], outs=[kvv_all[:].opt()])

                for m in range(6):
                    ps = qkp.tile([128, TLOC], F32, tag="qk")
                    for k in range(6):
                        nc.tensor.matmul(
                            ps, lhsT=wq_sb[:, k, m * 128:(m + 1) * 128],
                            rhs=hT[:, k, :], start=(k == 0), stop=(k == 5))
                    for par in range(2):
                        h = 2 * m + par
                        sl = slice(par * 64, par * 64 + 64)
                        if add_qk_bias:
                            nc.vector.tensor_scalar_add(
                                out=qT[sl, h, :], in0=ps[sl, :],
                                scalar1=bqk_sb[sl, 0, m:m + 1])
                        else:
                            nc.vector.tensor_copy(qT[sl, h, :], ps[sl, :])

            # ---------------- attention ----------------
            with tc.tile_pool(name="kch", bufs=1) as kchp, \
                 tc.tile_pool(name="vch", bufs=1) as vchp, \
                 tc.tile_pool(name="vaug", bufs=1) as vaugp, \
                 tc.tile_pool(name="ep", bufs=2, space="PSUM") as epp, \
                 tc.tile_pool(name="avp", bufs=2, space="PSUM") as avpp, \
                 tc.tile_pool(name="bcp", bufs=1, space="PSUM") as bcpp, \
                 tc.tile_pool(name="esb", bufs=6) as esbp:

                # gathered k: [r][6 ct][128][512]; v: [r][4 lt][128][768]
                k_ch = kchp.tile([128, 4, 6, TLOC], BF16)
                v_ch = vchp.tile([128, 4, 4, C], BF16)
                v_aug = vaugp.tile([128, NKT, 12 * 65], BF16)

                for r in range(4):
                    nc.sync.dma_start(
                        out=k_ch[:, r, :, :],
                        in_=kvk_all[r].rearrange("(ct p t) -> p ct t",
                                                 p=128, t=TLOC))
                for r in range(4):
                    nc.sync.dma_start(
                        out=v_ch[:, r, :, :],
                        in_=kvv_all[r].rearrange("(tt p c) -> p tt c",
                                                 p=128, c=C))
                # assemble v_aug: per rank, two chunk-pairs of v tiles
                va4 = v_aug[:].rearrange("p kt (h e) -> p kt h e", e=65)
                nc.vector.memset(va4[:, :, :, 64:65], 1.0)
                for r in range(4):
                    for half, kt0 in ((0, 2 * r), (1, 14 - 2 * r)):
                        vsrc = v_ch[:, r, 2 * half:2 * half + 2, :].rearrange(
                            "p l (h e) -> p l h e", e=64)
                        nc.vector.tensor_copy(
                            va4[:, kt0:kt0 + 2, :, 0:64], vsrc)

                def k_ap_of(kt, h):
                    ck = kt // 2
                    r = _rank_of_chunk(ck)
                    loc = _loc_of_chunk(ck) + (kt % 2) * 128
                    return k_ch[:, r, h // 2, loc:loc + 128]

                def finalize_head(h, pav):
                    nc.vector.tensor_copy(d_sb[0:1, :], pav[64:65, :])
                    pb = bcpp.tile([64, TLOC], F32, tag="bc", name="pbc")
                    nc.tensor.matmul(pb, lhsT=ones_pad, rhs=d_sb,
                                     start=True, stop=True)
                    b_sb = small.tile([64, TLOC], F32, tag="bsb", name="bsb")
                    nc.vector.reciprocal_approx_fast(out=b_sb, in_=pb)
                    nc.vector.tensor_mul(yT_all[0:64, h, :], pav[0:64, :], b_sb)

                def emit_av(pend):
                    h, e_sb, (g0, gn) = pend
                    pav = pavs[h]
                    off0 = _slot_off(g0)
                    for i in range(gn):
                        kt = g0 + i
                        w = _slot_w(kt)
                        so = _slot_off(kt) - off0
                        out = pav if w == 512 else pav[:, CHUNK:TLOC]
                        nc.tensor.matmul(
                            out, lhsT=v_aug[:, kt, h * 65:(h + 1) * 65],
                            rhs=e_sb[:, so:so + w],
                            start=(kt == 0), stop=(kt == NKT - 1),
                            skip_group_check=True)

                pavs = {}
                pends = []
                for h in range(12):
                    q_full = qT[:, h, :]
                    q_c1 = qT[:, h, CHUNK:TLOC]
                    pavs[h] = avpp.tile([65, TLOC], F32, tag="av",
                                        name=f"pav_{h}")
                    for (g0, gn) in GROUPS:
                        pe = epp.tile([128, 1024], F32, tag="e")
                        off0 = _slot_off(g0)
                        for i in range(gn):
                            kt = g0 + i
                            w = _slot_w(kt)
                            so = _slot_off(kt) - off0
                            nc.tensor.matmul(
                                pe[:, so:so + w], lhsT=k_ap_of(kt, h),
                                rhs=(q_full if w == 512 else q_c1),
                                start=True, stop=True)
                        e_sb = esbp.tile([128, 1024], BF16, tag="esb")
                        nc.scalar.activation(
                            out=e_sb, in_=pe,
                            func=mybir.ActivationFunctionType.Exp)
                        nc.vector.tensor_mul(
                            e_sb, e_sb, masks_sb[:, off0:off0 + 1024])
                        pends.append((h, e_sb, (g0, gn)))
                        if len(pends) > 4:
                            pend = pends.pop(0)
                            emit_av(pend)
                            if pend[2][0] + pend[2][1] == NKT:
                                finalize_head(pend[0], pavs[pend[0]])
                                del pavs[pend[0]]
                for pend in pends:
                    emit_av(pend)
                    if pend[2][0] + pend[2][1] == NKT:
                        finalize_head(pend[0], pavs[pend[0]])
                        del pavs[pend[0]]
                pends = []

            # ---------------- proj + residual + LN2 ----------------
            with tc.tile_pool(name="pp", bufs=2, space="PSUM") as ppp, \
                 tc.tile_pool(name="ln2", bufs=3) as ln2p, \
                 tc.tile_pool(name="tp2", bufs=2, space="PSUM") as tpp2:

                xn2s = []
                for t in range(4):
                    pp = ppp.tile([128, C], F32, tag="pp")
                    for h in range(12):
                        y_ap = yT_all[:, h, t * 128:(t + 1) * 128]
                        nc.tensor.matmul(pp[:, 0:512], lhsT=y_ap,
                                         rhs=wp_sb[:, h, 0:512],
                                         start=(h == 0), stop=(h == 11))
                        nc.tensor.matmul(pp[:, 512:768], lhsT=y_ap,
                                         rhs=wp_sb[:, h, 512:768],
                                         start=(h == 0), stop=(h == 11))
                    nc.vector.tensor_add(x_sb[:, t, :], x_sb[:, t, :], pp)
                    if add_proj_bias:
                        nc.vector.tensor_add(x_sb[:, t, :], x_sb[:, t, :],
                                             bout_sb[:, 0, :])
                    xn2 = ln2p.tile([128, C], BF16, tag="xn2", name="xn2")
                    layernorm_to(ln2p, x_sb[:, t, :], xn2, "2")
                    xn2s.append(xn2)
                for t in range(4):
                    for ct in range(6):
                        pt = tpp2.tile([128, 128], BF16, tag="tp2")
                        nc.tensor.transpose(
                            pt, xn2s[t][:, ct * 128:(ct + 1) * 128], ident)
                        nc.vector.tensor_copy(
                            hT[:, ct, t * 128:(t + 1) * 128], pt)

            # ---------------- MLP ----------------
            with tc.tile_pool(name="mlp", bufs=1) as mlpp, \
                 tc.tile_pool(name="wfc", bufs=6) as wfcp, \
                 tc.tile_pool(name="wfc2", bufs=6) as wfc2p, \
                 tc.tile_pool(name="osb", bufs=3) as osbp:

                gT = mlpp.tile([128, 24, TLOC], BF16)
                wfc_t = wfc_ext.ap().rearrange("(k p) n -> p k n", p=128)
                with tc.tile_pool(name="fcp", bufs=2, space="PSUM") as fcpp:
                    for m in range(24):
                        wt = wfcp.tile([128, 6, 128], BF16, tag="wfc")
                        nc.sync.dma_start(
                            out=wt, in_=wfc_t[:, :, m * 128:(m + 1) * 128])
                        pf = fcpp.tile([128, TLOC], F32, tag="fc")
                        for k in range(6):
                            nc.tensor.matmul(pf, lhsT=wt[:, k, :],
                                             rhs=hT[:, k, :],
                                             start=(k == 0), stop=(k == 5))
                        nc.scalar.activation(
                            out=gT[:, m, :], in_=pf,
                            func=mybir.ActivationFunctionType.Gelu_apprx_tanh,
                            bias=bfc_sb[:, m:m + 1])

                wfc2_t = wfc2_ext.ap().rearrange("(k p) n -> k p n", p=128)
                with tc.tile_pool(name="f2p", bufs=1, space="PSUM") as f2pp:
                    pf2s = [f2pp.tile([128, C], F32, tag=f"f2_{t}",
                                      name=f"pf2_{t}")
                            for t in range(4)]
                    for k in range(24):
                        wt2 = wfc2p.tile([128, C], BF16, tag="wfc2")
                        nc.sync.dma_start(out=wt2, in_=wfc2_t[k])
                        for t in range(4):
                            nc.tensor.matmul(
                                pf2s[t][:, 0:512],
                                lhsT=gT[:, k, t * 128:(t + 1) * 128],
                                rhs=wt2[:, 0:512],
                                start=(k == 0), stop=(k == 23))
                            nc.tensor.matmul(
                                pf2s[t][:, 512:768],
                                lhsT=gT[:, k, t * 128:(t + 1) * 128],
                                rhs=wt2[:, 512:768],
                                start=(k == 0), stop=(k == 23))
                    for t in range(4):
                        o_sb = osbp.tile([128, C], F32, tag="osb", name="osb")
                        nc.vector.tensor_add(o_sb, x_sb[:, t, :], pf2s[t])
                        if add_fc2_bias:
                            nc.vector.tensor_add(o_sb, o_sb, bout_sb[:, 1, :])
                        nc.sync.dma_start(
                            out=out_ext[t * 128:(t + 1) * 128, :], in_=o_sb)

    nc.compile()
    return nc


def _preprocess(inputs):
    f = lambda k: np.asarray(inputs[k], np.float32)
    x = f("x"); w_attn = f("w_attn"); b_attn = f("b_attn")
    w_proj = f("w_proj"); b_proj = f("b_proj")
    w_fc = f("w_fc"); b_fc = f("b_fc"); w_fc2 = f("w_fc2"); b_fc2 = f("b_fc2")
    ln1_g = f("ln1_g"); ln1_b = f("ln1_b"); ln2_g = f("ln2_g"); ln2_b = f("ln2_b")

    w_attn_eff = ln1_g[:, None] * w_attn
    b_attn_eff = b_attn + ln1_b @ w_attn
    s = 1.0 / np.sqrt(HD)
    w_q = w_attn_eff[:, 0:C] * s
    w_k = w_attn_eff[:, C:2 * C]
    w_v = w_attn_eff[:, 2 * C:3 * C]
    b_q = b_attn_eff[0:C] * s
    b_k = b_attn_eff[C:2 * C]
    b_v = b_attn_eff[2 * C:3 * C]
    b_proj_eff = b_proj + b_v @ w_proj
    w_fc_eff = ln2_g[:, None] * w_fc
    b_fc_eff = b_fc + ln2_b @ w_fc

    wq16 = np.ascontiguousarray(w_q.astype(BF))
    wk16 = np.ascontiguousarray(w_k.astype(BF))
    wv16 = np.ascontiguousarray(w_v.astype(BF))
    wp_pad = np.zeros((12, 128, C), np.float32)
    wp_pad[:, 0:64, :] = w_proj.reshape(12, 64, C)
    wp16 = np.ascontiguousarray(wp_pad.astype(BF))
    wfc16 = np.ascontiguousarray(w_fc_eff.astype(BF))
    wfc216 = np.ascontiguousarray(w_fc2.astype(BF))

    bqk = np.stack([b_q, b_k]).astype(np.float32)
    bout = np.stack([b_proj_eff, b_fc2]).astype(np.float32)

    flags = (bool(np.any(bqk != 0)), bool(np.any(b_proj_eff != 0)),
             bool(np.any(b_fc2 != 0)))

    # mask slab [128, 6144] per core group j; kt<8 slots cover both q-chunks
    kpos = np.arange(128)
    qpos = np.arange(CHUNK)
    masks = np.zeros((4, 128, MASK_W), np.float32)
    for j in range(4):
        for kt in range(NKT):
            gk = kt * 128 + kpos[:, None]
            off = _slot_off(kt)
            if kt < 8:
                gq0 = j * CHUNK + qpos[None, :]
                gq1 = (7 - j) * CHUNK + qpos[None, :]
                masks[j, :, off:off + 256] = (gq0 >= gk)
                masks[j, :, off + 256:off + 512] = (gq1 >= gk)
            else:
                gq1 = (7 - j) * CHUNK + qpos[None, :]
                masks[j, :, off:off + 256] = (gq1 >= gk)
    masks16 = masks.astype(BF)

    in_maps = []
    for c in range(NCORES):
        b, j = c // 4, c % 4
        x_loc = np.concatenate(
            [x[b, j * CHUNK:(j + 1) * CHUNK],
             x[b, (7 - j) * CHUNK:(8 - j) * CHUNK]]).astype(np.float32)
        in_maps.append({
            "x": np.ascontiguousarray(x_loc),
            "wq": wq16, "wk": wk16, "wv": wv16, "wp": wp16,
            "wfc": wfc16, "wfc2": wfc216,
            "masks": np.ascontiguousarray(masks16[j]),
            "bqk": bqk, "bfc": b_fc_eff.astype(np.float32), "bout": bout,
        })
    return in_maps, flags


def kernel(**inputs):
    global LAST_EXEC_NS, LAST_RESULTS
    in_maps, flags = _preprocess(inputs)
    if flags not in _CACHE:
        _CACHE[flags] = _build(*flags)
    nc = _CACHE[flags]
    trace = bool(os.environ.get("BASS_KERNEL_TRACE"))
    res = run_bass_kernel_spmd(nc, in_maps, core_ids=list(range(NCORES)),
                               trace=trace)
    LAST_EXEC_NS = res.exec_time_ns
    LAST_RESULTS = res
    out = np.empty((B, T, C), np.float32)
    for c in range(NCORES):
        b, j = c // 4, c % 4
        o = res.results[c]["out"]
        out[b, j * CHUNK:(j + 1) * CHUNK] = o[0:CHUNK]
        out[b, (7 - j) * CHUNK:(8 - j) * CHUNK] = o[CHUNK:TLOC]
    return out


# revision 21
# speedup vs baseline: 1.3983x; 1.0769x over previous
"""GPT-2 style transformer block on 8 TRN2 NeuronCores.

Sharding: token-data-parallel. Each batch's 2048 tokens are split into 8
chunks of 256; core c owns batch c//4 and chunks {j, 7-j} (j = c%4) so
causal attention work is balanced. QKV/proj/MLP/LN are purely local; the
only collectives are two AllGathers (k^T, then v) within each 4-core
batch group. Causality is enforced with per-core 0/1 mask tensors so all
cores run one identical SPMD graph (uniform loop bounds; masks zero the
beyond-causal tiles, which also makes the per-core graphs j-independent).

Matmuls run in bf16 (f32 PSUM accumulation); LN/softmax/residuals in f32.
LN affine params are folded into the following matmul weights host-side;
the attention 1/sqrt(hd) scale is folded into w_q; the v-bias is folded
into the proj bias via the softmax-rows-sum-to-one identity. Softmax is
computed without max-subtraction (scores are O(1) here, exp cannot
overflow in f32) as exp(s) normalized by a denominator obtained for free
as an extra ones-column in the av matmul. Both local q-chunks share one
[65, 512] av accumulator per head; key tiles 0..7 are scored against all
512 local queries in one matmul, tiles 8..15 only against q-chunk 1.
"""

import os
import sys

sys.path.insert(0, "/opt/trn_rl_repo")

import numpy as np
import ml_dtypes

import concourse.bass as bass
import concourse.tile as tile
from concourse import bacc, mybir
from concourse.bass_utils import run_bass_kernel_spmd
from concourse.masks import make_identity

F32 = mybir.dt.float32
FP8 = mybir.dt.float8e4
BF16 = mybir.dt.bfloat16
BF = ml_dtypes.bfloat16

B, T, C, H, HD = 2, 2048, 768, 12, 64
EPS = 1e-5
NCORES = 8
CHUNK = 256            # global chunk size (tokens)
TLOC = 512             # local tokens per core (2 chunks)
NKT = T // 128         # 16 key tiles per batch
CC = T // CHUNK        # 8 chunks per batch

# e-slot layout: kt<8 -> 512 wide (both q-chunks), kt>=8 -> 256 (q-chunk 1)
def _slot_off(kt):
    return kt * 512 if kt < 8 else 4096 + (kt - 8) * 256


def _slot_w(kt):
    return 512 if kt < 8 else 256


MASK_W = 8 * 512 + 8 * 256   # 6144
# exp groups: contiguous 1024-col spans of the slot layout
GROUPS = [(0, 2), (2, 2), (4, 2), (6, 2), (8, 4), (12, 4)]

KT_ELEMS = 6 * 128 * TLOC          # k^T bounce: [6 ct][128 p][512 t]
V_ELEMS = 4 * 128 * C              # v bounce:   [4 tt][128 p][768 c]

LAST_EXEC_NS = None
LAST_RESULTS = None
_CACHE = {}


def _rank_of_chunk(ck):
    return ck if ck < 4 else 7 - ck


def _loc_of_chunk(ck):
    return 0 if ck < 4 else CHUNK


def _build(add_qk_bias, add_proj_bias, add_fc2_bias):
    nc = bacc.Bacc("TRN2", target_bir_lowering=False, debug=False,
                   num_devices=NCORES)

    x_ext = nc.dram_tensor("x", [TLOC, C], F32, kind="ExternalInput")
    wq_ext = nc.dram_tensor("wq", [C, C], BF16, kind="ExternalInput")
    wk_ext = nc.dram_tensor("wk", [C, C], BF16, kind="ExternalInput")
    wv_ext = nc.dram_tensor("wv", [C, C], BF16, kind="ExternalInput")
    wp_ext = nc.dram_tensor("wp", [12, 128, C], BF16, kind="ExternalInput")
    wfc_ext = nc.dram_tensor("wfc", [C, 4 * C], BF16, kind="ExternalInput")
    wfc2_ext = nc.dram_tensor("wfc2", [4 * C, C], BF16, kind="ExternalInput")
    masks_ext = nc.dram_tensor("masks", [128, MASK_W], BF16,
                               kind="ExternalInput")
    bqk_ext = nc.dram_tensor("bqk", [2, C], F32, kind="ExternalInput")
    bfc_ext = nc.dram_tensor("bfc", [4 * C], F32, kind="ExternalInput")
    bout_ext = nc.dram_tensor("bout", [2, C], F32, kind="ExternalInput")
    out_ext = nc.dram_tensor("out", [TLOC, C], F32, kind="ExternalOutput")

    with tile.TileContext(nc) as tc:
        with tc.tile_pool(name="dram", bufs=1, space="DRAM") as dram, \
             tc.tile_pool(name="singles", bufs=1) as singles, \
             tc.tile_pool(name="persist", bufs=1) as persist, \
             tc.tile_pool(name="small", bufs=3) as small:

            kvk_in = dram.tile([KT_ELEMS], FP8)
            kvk_all = dram.tile([4, KT_ELEMS], FP8)
            kvv_in = dram.tile([V_ELEMS], FP8)
            kvv_all = dram.tile([4, V_ELEMS], FP8)

            ident = singles.tile([128, 128], BF16)
            make_identity(nc, ident)
            eps_sb = singles.tile([128, 1], F32)
            nc.vector.memset(eps_sb, EPS)
            ones_pad = singles.tile([128, 64], F32)
            nc.vector.memset(ones_pad, 0.0)
            nc.vector.memset(ones_pad[0:1, :], 1.0)
            d_sb = singles.tile([128, TLOC], F32)
            nc.vector.memset(d_sb, 1.0)

            wq_sb = persist.tile([128, 6, C], BF16)
            wk_sb = persist.tile([128, 6, C], BF16)
            wv_sb = persist.tile([128, 6, C], BF16)
            wp_sb = persist.tile([128, 12, C], BF16)
            for sb, ext in ((wq_sb, wq_ext), (wk_sb, wk_ext),
                            (wv_sb, wv_ext)):
                nc.sync.dma_start(
                    out=sb, in_=ext.ap().rearrange("(ct p) c -> p ct c", p=128))
            nc.sync.dma_start(
                out=wp_sb, in_=wp_ext.ap().rearrange("h p c -> p h c"))

            bqk_sb = singles.tile([128, 2, 6], F32)
            if add_qk_bias:
                nc.sync.dma_start(
                    out=bqk_sb,
                    in_=bqk_ext.ap().rearrange("b (m p) -> p b m", p=128))
            bfc_sb = singles.tile([128, 24], F32)
            nc.sync.dma_start(
                out=bfc_sb, in_=bfc_ext.ap().rearrange("(m p) -> p m", p=128))
            bout_sb = singles.tile([128, 2, C], F32)
            if add_proj_bias or add_fc2_bias:
                bc = bout_ext.ap()
                nc.sync.dma_start(
                    out=bout_sb,
                    in_=bass.AP(tensor=bc.tensor, offset=bc.offset,
                                ap=[[0, 128], bc.ap[0], bc.ap[1]]))

            masks_sb = persist.tile([128, MASK_W], BF16)
            nc.sync.dma_start(out=masks_sb, in_=masks_ext.ap())

            x_sb = persist.tile([128, 4, C], F32)     # local x, becomes xmid
            hT = persist.tile([128, 6, TLOC], BF16)   # h^T, reused for h2^T
            qT = persist.tile([128, 12, TLOC], BF16)
            nc.vector.memset(qT[:], 0.0)
            yT_all = persist.tile([128, 12, TLOC], BF16)
            nc.vector.memset(yT_all[64:128, :, :], 0.0)

            def layernorm_to(pool, xt, dst, tagsuf):
                stats = pool.tile([128, 3, 6], F32, tag="st" + tagsuf,
                                  name="st" + tagsuf)
                for sg in range(3):
                    nc.vector.bn_stats(out=stats[:, sg, :],
                                       in_=xt[:, sg * 256:(sg + 1) * 256])
                mv = pool.tile([128, 2], F32, tag="mv" + tagsuf,
                               name="mv" + tagsuf)
                nc.vector.bn_aggr(out=mv, in_=stats)
                nc.scalar.activation(out=mv[:, 1:2], in_=mv[:, 1:2],
                                     func=mybir.ActivationFunctionType.Sqrt,
                                     bias=eps_sb)
                nc.vector.reciprocal(out=mv[:, 1:2], in_=mv[:, 1:2])
                nc.vector.tensor_scalar(out=dst, in0=xt,
                                        scalar1=mv[:, 0:1], scalar2=mv[:, 1:2],
                                        op0=mybir.AluOpType.subtract,
                                        op1=mybir.AluOpType.mult)

            # ---------------- LN1 + transpose + QKV + AGs ----------------
            with tc.tile_pool(name="ln", bufs=3) as lnp, \
                 tc.tile_pool(name="tp", bufs=2, space="PSUM") as tpp, \
                 tc.tile_pool(name="qkp", bufs=2, space="PSUM") as qkp, \
                 tc.tile_pool(name="vp", bufs=2, space="PSUM") as vpp, \
                 tc.tile_pool(name="vsb", bufs=1) as vsbp:

                kT = vsbp.tile([128, 6, TLOC], FP8)
                for t in range(4):
                    nc.sync.dma_start(out=x_sb[:, t, :],
                                      in_=x_ext[t * 128:(t + 1) * 128, :])
                    xn = lnp.tile([128, C], BF16, tag="xn")
                    layernorm_to(lnp, x_sb[:, t, :], xn, "1")
                    for ct in range(6):
                        pt = tpp.tile([128, 128], BF16, tag="tp")
                        nc.tensor.transpose(
                            pt, xn[:, ct * 128:(ct + 1) * 128], ident)
                        nc.vector.tensor_copy(
                            hT[:, ct, t * 128:(t + 1) * 128], pt)

                # k^T first: it feeds the first collective
                for m in range(6):
                    ps = qkp.tile([128, TLOC], F32, tag="qk")
                    for k in range(6):
                        nc.tensor.matmul(
                            ps, lhsT=wk_sb[:, k, m * 128:(m + 1) * 128],
                            rhs=hT[:, k, :], start=(k == 0), stop=(k == 5))
                    if add_qk_bias:
                        nc.vector.tensor_scalar_add(
                            out=kT[:, m, :], in0=ps,
                            scalar1=bqk_sb[:, 1, m:m + 1])
                    else:
                        nc.vector.tensor_copy(kT[:, m, :], ps)
                nc.sync.dma_start(
                    out=kvk_in[:].rearrange("(ct p t) -> p ct t", p=128, t=TLOC),
                    in_=kT)
                nc.gpsimd.collective_compute(
                    "AllGather", mybir.AluOpType.bypass,
                    replica_groups=[[0, 1, 2, 3], [4, 5, 6, 7]],
                    ins=[kvk_in[:].opt()], outs=[kvk_all[:].opt()])

                v_sb = vsbp.tile([128, 4, C], FP8)
                for tt in range(4):
                    pv = vpp.tile([128, C], F32, tag="v")
                    for k in range(6):
                        nc.tensor.matmul(
                            pv[:, 0:512],
                            lhsT=hT[:, k, tt * 128:(tt + 1) * 128],
                            rhs=wv_sb[:, k, 0:512],
                            start=(k == 0), stop=(k == 5))
                        nc.tensor.matmul(
                            pv[:, 512:768],
                            lhsT=hT[:, k, tt * 128:(tt + 1) * 128],
                            rhs=wv_sb[:, k, 512:768],
                            start=(k == 0), stop=(k == 5))
                    nc.vector.tensor_copy(v_sb[:, tt, :], pv)
                nc.sync.dma_start(
                    out=kvv_in[:].rearrange("(tt p c) -> p tt c", p=128, c=C),
                    in_=v_sb)
                nc.gpsimd.collective_compute(
                    "AllGather", mybir.AluOpType.bypass,
                    replica_groups=[[0, 1, 2, 3], [4, 5, 6, 7]],
                    ins=[kvv_in[:].opt()], outs=[kvv_all[:].opt()])

                for m in range(6):
                    ps = qkp.tile([128, TLOC], F32, tag="qk")
                    for k in range(6):
                        nc.tensor.matmul(
                            ps, lhsT=wq_sb[:, k, m * 128:(m + 1) * 128],
                            rhs=hT[:, k, :], start=(k == 0), stop=(k == 5))
                    for par in range(2):
                        h = 2 * m + par
                        sl = slice(par * 64, par * 64 + 64)
                        if add_qk_bias:
                            nc.vector.tensor_scalar_add(
                                out=qT[sl, h, :], in0=ps[sl, :],
                                scalar1=bqk_sb[sl, 0, m:m + 1])
                        else:
                            nc.vector.tensor_copy(qT[sl, h, :], ps[sl, :])

            # ---------------- attention ----------------
            with tc.tile_pool(name="kch", bufs=1) as kchp, \
                 tc.tile_pool(name="vch", bufs=1) as vchp, \
                 tc.tile_pool(name="vaug", bufs=1) as vaugp, \
                 tc.tile_pool(name="ep", bufs=2, space="PSUM") as epp, \
                 tc.tile_pool(name="avp", bufs=2, space="PSUM") as avpp, \
                 tc.tile_pool(name="bcp", bufs=1, space="PSUM") as bcpp, \
                 tc.tile_pool(name="esb", bufs=6) as esbp:

                # gathered k: [r][6 ct][128][512]; v: [r][4 lt][128][768]
                k_f8 = kchp.tile([128, 4, 6, TLOC], FP8)
                k_ch = kchp.tile([128, 4, 6, TLOC], BF16)
                v_ch = vchp.tile([128, 4, 4, C], FP8)
                v_aug = vaugp.tile([128, NKT, 12 * 65], BF16)

                for r in range(4):
                    nc.sync.dma_start(
                        out=k_f8[:, r, :, :],
                        in_=kvk_all[r].rearrange("(ct p t) -> p ct t",
                                                 p=128, t=TLOC))
                    nc.vector.tensor_copy(k_ch[:, r, :, :], k_f8[:, r, :, :])
                for r in range(4):
                    nc.sync.dma_start(
                        out=v_ch[:, r, :, :],
                        in_=kvv_all[r].rearrange("(tt p c) -> p tt c",
                                                 p=128, c=C))
                # assemble v_aug: per rank, two chunk-pairs of v tiles
                va4 = v_aug[:].rearrange("p kt (h e) -> p kt h e", e=65)
                nc.vector.memset(va4[:, :, :, 64:65], 1.0)
                for r in range(4):
                    for half, kt0 in ((0, 2 * r), (1, 14 - 2 * r)):
                        vsrc = v_ch[:, r, 2 * half:2 * half + 2, :].rearrange(
                            "p l (h e) -> p l h e", e=64)
                        nc.vector.tensor_copy(
                            va4[:, kt0:kt0 + 2, :, 0:64], vsrc)

                def k_ap_of(kt, h):
                    ck = kt // 2
                    r = _rank_of_chunk(ck)
                    loc = _loc_of_chunk(ck) + (kt % 2) * 128
                    return k_ch[:, r, h // 2, loc:loc + 128]

                def finalize_head(h, pav):
                    nc.vector.tensor_copy(d_sb[0:1, :], pav[64:65, :])
                    pb = bcpp.tile([64, TLOC], F32, tag="bc", name="pbc")
                    nc.tensor.matmul(pb, lhsT=ones_pad, rhs=d_sb,
                                     start=True, stop=True)
                    b_sb = small.tile([64, TLOC], F32, tag="bsb", name="bsb")
                    nc.vector.reciprocal_approx_fast(out=b_sb, in_=pb)
                    nc.vector.tensor_mul(yT_all[0:64, h, :], pav[0:64, :], b_sb)

                def emit_av(pend):
                    h, e_sb, (g0, gn) = pend
                    pav = pavs[h]
                    off0 = _slot_off(g0)
                    for i in range(gn):
                        kt = g0 + i
                        w = _slot_w(kt)
                        so = _slot_off(kt) - off0
                        out = pav if w == 512 else pav[:, CHUNK:TLOC]
                        nc.tensor.matmul(
                            out, lhsT=v_aug[:, kt, h * 65:(h + 1) * 65],
                            rhs=e_sb[:, so:so + w],
                            start=(kt == 0), stop=(kt == NKT - 1),
                            skip_group_check=True)

                pavs = {}
                pends = []
                for h in range(12):
                    q_full = qT[:, h, :]
                    q_c1 = qT[:, h, CHUNK:TLOC]
                    pavs[h] = avpp.tile([65, TLOC], F32, tag="av",
                                        name=f"pav_{h}")
                    for (g0, gn) in GROUPS:
                        pe = epp.tile([128, 1024], F32, tag="e")
                        off0 = _slot_off(g0)
                        for i in range(gn):
                            kt = g0 + i
                            w = _slot_w(kt)
                            so = _slot_off(kt) - off0
                            nc.tensor.matmul(
                                pe[:, so:so + w], lhsT=k_ap_of(kt, h),
                                rhs=(q_full if w == 512 else q_c1),
                                start=True, stop=True)
                        e_sb = esbp.tile([128, 1024], BF16, tag="esb")
                        nc.scalar.activation(
                            out=e_sb, in_=pe,
                            func=mybir.ActivationFunctionType.Exp)
                        nc.vector.tensor_mul(
                            e_sb, e_sb, masks_sb[:, off0:off0 + 1024])
                        pends.append((h, e_sb, (g0, gn)))
                        if len(pends) > 4:
                            pend = pends.pop(0)
                            emit_av(pend)
                            if pend[2][0] + pend[2][1] == NKT:
                                finalize_head(pend[0], pavs[pend[0]])
                                del pavs[pend[0]]
                for pend in pends:
                    emit_av(pend)
                    if pend[2][0] + pend[2][1] == NKT:
                        finalize_head(pend[0], pavs[pend[0]])
                        del pavs[pend[0]]
                pends = []

            # ---------------- proj + residual + LN2 ----------------
            with tc.tile_pool(name="pp", bufs=2, space="PSUM") as ppp, \
                 tc.tile_pool(name="ln2", bufs=3) as ln2p, \
                 tc.tile_pool(name="tp2", bufs=2, space="PSUM") as tpp2:

                xn2s = []
                for t in range(4):
                    pp = ppp.tile([128, C], F32, tag="pp")
                    for h in range(12):
                        y_ap = yT_all[:, h, t * 128:(t + 1) * 128]
                        nc.tensor.matmul(pp[:, 0:512], lhsT=y_ap,
                                         rhs=wp_sb[:, h, 0:512],
                                         start=(h == 0), stop=(h == 11))
                        nc.tensor.matmul(pp[:, 512:768], lhsT=y_ap,
                                         rhs=wp_sb[:, h, 512:768],
                                         start=(h == 0), stop=(h == 11))
                    nc.vector.tensor_add(x_sb[:, t, :], x_sb[:, t, :], pp)
                    if add_proj_bias:
                        nc.vector.tensor_add(x_sb[:, t, :], x_sb[:, t, :],
                                             bout_sb[:, 0, :])
                    xn2 = ln2p.tile([128, C], BF16, tag="xn2", name="xn2")
                    layernorm_to(ln2p, x_sb[:, t, :], xn2, "2")
                    xn2s.append(xn2)
                for t in range(4):
                    for ct in range(6):
                        pt = tpp2.tile([128, 128], BF16, tag="tp2")
                        nc.tensor.transpose(
                            pt, xn2s[t][:, ct * 128:(ct + 1) * 128], ident)
                        nc.vector.tensor_copy(
                            hT[:, ct, t * 128:(t + 1) * 128], pt)

            # ---------------- MLP ----------------
            with tc.tile_pool(name="mlp", bufs=1) as mlpp, \
                 tc.tile_pool(name="wfc", bufs=6) as wfcp, \
                 tc.tile_pool(name="wfc2", bufs=6) as wfc2p, \
                 tc.tile_pool(name="osb", bufs=3) as osbp:

                gT = mlpp.tile([128, 24, TLOC], BF16)
                wfc_t = wfc_ext.ap().rearrange("(k p) n -> p k n", p=128)
                with tc.tile_pool(name="fcp", bufs=2, space="PSUM") as fcpp:
                    for m in range(24):
                        wt = wfcp.tile([128, 6, 128], BF16, tag="wfc")
                        nc.sync.dma_start(
                            out=wt, in_=wfc_t[:, :, m * 128:(m + 1) * 128])
                        pf = fcpp.tile([128, TLOC], F32, tag="fc")
                        for k in range(6):
                            nc.tensor.matmul(pf, lhsT=wt[:, k, :],
                                             rhs=hT[:, k, :],
                                             start=(k == 0), stop=(k == 5))
                        nc.scalar.activation(
                            out=gT[:, m, :], in_=pf,
                            func=mybir.ActivationFunctionType.Gelu_apprx_tanh,
                            bias=bfc_sb[:, m:m + 1])

                wfc2_t = wfc2_ext.ap().rearrange("(k p) n -> k p n", p=128)
                with tc.tile_pool(name="f2p", bufs=1, space="PSUM") as f2pp:
                    pf2s = [f2pp.tile([128, C], F32, tag=f"f2_{t}",
                                      name=f"pf2_{t}")
                            for t in range(4)]
                    for k in range(24):
                        wt2 = wfc2p.tile([128, C], BF16, tag="wfc2")
                        nc.sync.dma_start(out=wt2, in_=wfc2_t[k])
                        for t in range(4):
                            nc.tensor.matmul(
                                pf2s[t][:, 0:512],
                                lhsT=gT[:, k, t * 128:(t + 1) * 128],
                                rhs=wt2[:, 0:512],
                                start=(k == 0), stop=(k == 23))
                            nc.tensor.matmul(
                                pf2s[t][:, 512:768],
                                lhsT=gT[:, k, t * 128:(t + 1) * 128],
                                rhs=wt2[:, 512:768],
                                start=(k == 0), stop=(k == 23))
                    for t in range(4):
                        o_sb = osbp.tile([128, C], F32, tag="osb", name="osb")
                        nc.vector.tensor_add(o_sb, x_sb[:, t, :], pf2s[t])
                        if add_fc2_bias:
                            nc.vector.tensor_add(o_sb, o_sb, bout_sb[:, 1, :])
                        nc.sync.dma_start(
                            out=out_ext[t * 128:(t + 1) * 128, :], in_=o_sb)

    nc.compile()
    return nc


def _preprocess(inputs):
    f = lambda k: np.asarray(inputs[k], np.float32)
    x = f("x"); w_attn = f("w_attn"); b_attn = f("b_attn")
    w_proj = f("w_proj"); b_proj = f("b_proj")
    w_fc = f("w_fc"); b_fc = f("b_fc"); w_fc2 = f("w_fc2"); b_fc2 = f("b_fc2")
    ln1_g = f("ln1_g"); ln1_b = f("ln1_b"); ln2_g = f("ln2_g"); ln2_b = f("ln2_b")

    w_attn_eff = ln1_g[:, None] * w_attn
    b_attn_eff = b_attn + ln1_b @ w_attn
    s = 1.0 / np.sqrt(HD)
    w_q = w_attn_eff[:, 0:C] * s
    w_k = w_attn_eff[:, C:2 * C]
    w_v = w_attn_eff[:, 2 * C:3 * C]
    b_q = b_attn_eff[0:C] * s
    b_k = b_attn_eff[C:2 * C]
    b_v = b_attn_eff[2 * C:3 * C]
    b_proj_eff = b_proj + b_v @ w_proj
    w_fc_eff = ln2_g[:, None] * w_fc
    b_fc_eff = b_fc + ln2_b @ w_fc

    wq16 = np.ascontiguousarray(w_q.astype(BF))
    wk16 = np.ascontiguousarray(w_k.astype(BF))
    wv16 = np.ascontiguousarray(w_v.astype(BF))
    wp_pad = np.zeros((12, 128, C), np.float32)
    wp_pad[:, 0:64, :] = w_proj.reshape(12, 64, C)
    wp16 = np.ascontiguousarray(wp_pad.astype(BF))
    wfc16 = np.ascontiguousarray(w_fc_eff.astype(BF))
    wfc216 = np.ascontiguousarray(w_fc2.astype(BF))

    bqk = np.stack([b_q, b_k]).astype(np.float32)
    bout = np.stack([b_proj_eff, b_fc2]).astype(np.float32)

    flags = (bool(np.any(bqk != 0)), bool(np.any(b_proj_eff != 0)),
             bool(np.any(b_fc2 != 0)))

    # mask slab [128, 6144] per core group j; kt<8 slots cover both q-chunks
    kpos = np.arange(128)
    qpos = np.arange(CHUNK)
    masks = np.zeros((4, 128, MASK_W), np.float32)
    for j in range(4):
        for kt in range(NKT):
            gk = kt * 128 + kpos[:, None]
            off = _slot_off(kt)
            if kt < 8:
                gq0 = j * CHUNK + qpos[None, :]
                gq1 = (7 - j) * CHUNK + qpos[None, :]
                masks[j, :, off:off + 256] = (gq0 >= gk)
                masks[j, :, off + 256:off + 512] = (gq1 >= gk)
            else:
                gq1 = (7 - j) * CHUNK + qpos[None, :]
                masks[j, :, off:off + 256] = (gq1 >= gk)
    masks16 = masks.astype(BF)

    in_maps = []
    for c in range(NCORES):
        b, j = c // 4, c % 4
        x_loc = np.concatenate(
            [x[b, j * CHUNK:(j + 1) * CHUNK],
             x[b, (7 - j) * CHUNK:(8 - j) * CHUNK]]).astype(np.float32)
        in_maps.append({
            "x": np.ascontiguousarray(x_loc),
            "wq": wq16, "wk": wk16, "wv": wv16, "wp": wp16,
            "wfc": wfc16, "wfc2": wfc216,
            "masks": np.ascontiguousarray(masks16[j]),
            "bqk": bqk, "bfc": b_fc_eff.astype(np.float32), "bout": bout,
        })
    return in_maps, flags


def kernel(**inputs):
    global LAST_EXEC_NS, LAST_RESULTS
    in_maps, flags = _preprocess(inputs)
    if flags not in _CACHE:
        _CACHE[flags] = _build(*flags)
    nc = _CACHE[flags]
    trace = bool(os.environ.get("BASS_KERNEL_TRACE"))
    res = run_bass_kernel_spmd(nc, in_maps, core_ids=list(range(NCORES)),
                               trace=trace)
    LAST_EXEC_NS = res.exec_time_ns
    LAST_RESULTS = res
    out = np.empty((B, T, C), np.float32)
    for c in range(NCORES):
        b, j = c // 4, c % 4
        o = res.results[c]["out"]
        out[b, j * CHUNK:(j + 1) * CHUNK] = o[0:CHUNK]
        out[b, (7 - j) * CHUNK:(8 - j) * CHUNK] = o[CHUNK:TLOC]
    return out
